# revision 41
# baseline (speedup 1.0000x reference)
"""CNN-LSTM decoder Trainium2 kernel (8 NeuronCores, data-parallel over batch).

Model (see reference): Conv1d(256->256,k=5,p=2) -> BatchNorm1d(train stats) ->
ReLU -> LSTM(256->512, T=1024) -> Linear(512->64) -> ReLU -> Linear(64->2).

Distribution: batch 128 split 16/core across 8 cores.  Per core:

  Conv:    time-block tiled ([128, w+4] x slices, N=w bf16 matmuls,
           BN+ReLU fused into the ACT eviction straight into a persistent
           SBUF buffer featsT[cc][ci, t, b] in bf16).  Only the first 32
           output cols are computed before the scan starts; the rest is
           emitted into the scan loop against per-piece consumption
           deadlines and hidden in engine idle gaps by the Tile priority
           scheduler.  BN batch stats are computed host-side.

  Scan:    fully transposed layout.  Gates live as gatesT [128 gate-dims,
           16 batch] PSUM tiles, two halves (H-slices 0-1 / 2-3, one PSUM
           bank each); psum cols = [i|f|g|o]*32 + s2*16 + b.  Per step:
             - 2 bias matmuls (K=8 mask trick) open each half's accum group
             - 32 input-projection matmuls (wihT chunks @ featsT[:, t, :]),
               emitted one step ahead so they fill PE idle time
             - 64 recurrent matmuls (whhT chunks @ hT k-slices), N=16 bf16
           Tail per half (TAIL_SCAN): one sigmoid over i/f/g writing to
           EVEN columns of the sig tile (odd columns preset to zero; g rows
           pre-scaled x2 so tanh(x) = 2*sigmoid(2x)-1), sigmoid over o,
           then u = (sig2g-.5)*sigi into the even cols of an interleaved
           [., c~, u, c~, u, ...] buffer and the whole c-update
           c~_j = sigf_j*c~_j + u_j as ONE DVE tensor_tensor_scan
           (2-element cells [reset, apply]; d0 = [0|sigf] interleaved via
           the preset zeros, ping-pong buffers per step parity).  c~ = c/2,
           so sigma(2c) = sigma(4c~).  h = (sig(4c~)-.5)*sigo is written
           as h/2 in bf16, the exact moving operand of the next step's
           recurrent matmuls.  Same-engine RAW deps in the tail are demoted
           to nosync edges (_demote): in-order engines make the semaphores
           redundant, removing their latency from the recurrence chain.
           The half whose tail finished late leads the next step
           (alternating), keeping both ACT chains near-saturated.  The
           cycle is bound by the dense ACT sequence (2 sigma96 + 2 sigma_o
           + 2 sigma(4c~)) plus the psum-stop coupling.

  Head:    transposed too: hid [64, 16] = relu(w1 @ h + b1); out [2, 16].

All host-side weight massaging (permutation, scaling, transposes, BN stats)
happens in kernel() below; the device kernel is compiled once per process.
"""

import sys

sys.path.insert(0, "/opt/trn_rl_repo")

import ml_dtypes
import numpy as np

import concourse.bass as bass
import concourse.tile as tile
from concourse import bacc, mybir
from concourse.bass_utils import run_bass_kernel_spmd

F32 = mybir.dt.float32
F32R = mybir.dt.float32r
BF16 = mybir.dt.bfloat16
AF = mybir.ActivationFunctionType
OP = mybir.AluOpType

B, C, T, H = 128, 256, 1024, 512
G = 4 * H  # 2048
NCORES = 8
BL = B // NCORES  # 16 examples per core
EPS = 1e-5

# gate chunk m = gt*4 + s (gt in [i,f,o,g], s = H-slice 0..3)
# psum half = s // 2; col within half = gt*32 + (s % 2)*16 + b
# within each half, o-gate chunks (gt==2 in perm order) go last so the
# ifg sigmoid is not gated on the o-column matmuls
A_CHUNKS = ([m for m in range(16) if (m % 4) < 2 and m // 4 != 2]
            + [m for m in range(16) if (m % 4) < 2 and m // 4 == 2])
B_CHUNKS = ([m for m in range(16) if (m % 4) >= 2 and m // 4 != 2]
            + [m for m in range(16) if (m % 4) >= 2 and m // 4 == 2])

PSUM_PAD = True
ALT_LEAD = True
HH_SPLIT = True
C_FRESH = False
POOL_V = 1
KORD = 0
SGROUP = 0
EVICT_DVE = False
CPS_BUFS = 3
DL_MARGIN = 0
HT_BUFS = 2
SIG_BUFS = 2
PS_BUFS = 2
PRE0 = 32
SIG_SPLIT = 2
CONV_STREAM = True
CONV_BF16 = True
DEMOTE = True   # demote same-engine data deps to nosync (engine order
                # guarantees RAW on HW: in-order engines drain the pipe
                # between instructions)
TAIL_DVE = True   # (pre-scan fallback) v on DVE ahead of u/c, engine-chained
TAIL_SCAN = True  # c-update as one tensor_tensor_scan over the interleaved
                  # [c~|u] ping-pong buffers (c~ = c/2)
SIG_MERGE = False   # (tried) merge sigma_ifg+sigma_o: breaks phase stagger
PS_ONE = False      # (tried) both halves in one psum bank: re-phases badly
SIG_STOPDEP = False
SCO_FUSE = False    # (tried) fuse sigma_o into sigma(4c~): o-copy lands on
                    # the DVE critical chain

_cache = {}


def _demote(cons, *prods):
    """Demote cons's sync deps on prods to nosync deps.

    Only valid when cons and every prod run on the same engine: the
    scheduler still enforces ordering, and in-order engine execution
    makes the RAW safe without a semaphore."""
    if not DEMOTE:
        return
    ci = cons.ins
    sync = ci.sync_dependency_set_copy()
    for p in prods:
        pi = p.ins
        assert pi.engine == ci.engine, (pi.engine, ci.engine)
        if pi.name in sync:
            ci.remove_dependency(pi.name)
            ci.add_dependency(
                pi.name, mybir.DependencyInfo(sync=False, no_sync=True)
            )


GT2COL = (0, 1, 3, 2)  # perm order [i,f,o,g] -> col blocks [i,f,g,o]


def _mm_dest(psA, psB, m):
    gt, s = m // 4, m % 4
    if PS_ONE:
        c0 = 128 * (s // 2) + GT2COL[gt] * 32 + (s % 2) * 16
        return psA, c0
    ps = psA if s < 2 else psB
    c0 = GT2COL[gt] * 32 + (s % 2) * 16
    return ps, c0


def _build(nT=T, skip_conv=False):
    CDT = BF16 if CONV_BF16 else F32R
    nc = bacc.Bacc("TRN2", target_bir_lowering=False, debug=False,
                   num_devices=NCORES)

    x_l = nc.dram_tensor("x_l", [BL, C, T], CDT, kind="ExternalInput").ap()
    cw = nc.dram_tensor("cw", [10, 128, 256], CDT, kind="ExternalInput").ap()
    bn_ab = nc.dram_tensor("bn_ab", [C, 2], F32, kind="ExternalInput").ap()
    wihT = nc.dram_tensor("wihT", [2, 128, G], BF16, kind="ExternalInput").ap()
    whhT = nc.dram_tensor("whhT", [4, 128, G], BF16, kind="ExternalInput").ap()
    biasT = nc.dram_tensor("biasT", [2, 8, 128], BF16, kind="ExternalInput").ap()
    maskb = nc.dram_tensor("maskb", [8, 128], BF16, kind="ExternalInput").ap()
    biasTf = nc.dram_tensor("biasTf", [16, 128], BF16, kind="ExternalInput").ap()
    maskf = nc.dram_tensor("maskf", [16, 256], BF16, kind="ExternalInput").ap()
    w1T = nc.dram_tensor("w1T", [4, 128, 64], BF16, kind="ExternalInput").ap()
    b1r = nc.dram_tensor("b1r", [1, 64], BF16, kind="ExternalInput").ap()
    w2T = nc.dram_tensor("w2T", [64, 2], BF16, kind="ExternalInput").ap()
    b2r = nc.dram_tensor("b2r", [1, 2], BF16, kind="ExternalInput").ap()
    ones1 = nc.dram_tensor("ones1", [1, 16], BF16, kind="ExternalInput").ap()

    out = nc.dram_tensor("out", [2, BL], F32, kind="ExternalOutput").ap()

    if SCO_FUSE:
        # o-gate chunks first: their psum cols finish early so the DVE
        # o-copies run during the whh block, fully hidden
        A_CH = ([m for m in range(16) if (m % 4) < 2 and m // 4 == 2]
                + [m for m in range(16) if (m % 4) < 2 and m // 4 != 2])
        B_CH = ([m for m in range(16) if (m % 4) >= 2 and m // 4 == 2]
                + [m for m in range(16) if (m % 4) >= 2 and m // 4 != 2])
    else:
        A_CH, B_CH = A_CHUNKS, B_CHUNKS

    with tile.TileContext(nc) as tc:
        with (
            tc.tile_pool(name="const", bufs=1) as const,
            tc.tile_pool(name="state", bufs=1) as state,
        ):
            # ---- persistent constants in SBUF ----
            cwb = const.tile([128, 10 * 256], CDT, tag="cwb")
            cw_sb = [cwb[:, 256 * i:256 * (i + 1)] for i in range(10)]
            nc.sync.dma_start(
                cwb[:].rearrange("p (n c) -> p n c", n=10),
                cw[0:10].transpose([1, 0, 2]),
            )
            bn_sb = [const.tile([128, 2], F32, name=f"bn{i}", tag=f"bn{i}")
                     for i in range(2)]
            for i in range(2):
                nc.sync.dma_start(bn_sb[i][:], bn_ab[128 * i:128 * (i + 1), :])
            # (DMAs for the tensors below are emitted after the conv
            # prelude so the prelude's x slices go first in the DMA queue;
            # they only have to arrive before scan step 0's matmuls.)
            wihb = const.tile([128, 2 * G], BF16, tag="wihb")
            wih_sb = [wihb[:, G * i:G * (i + 1)] for i in range(2)]
            whhb = const.tile([128, 4 * G], BF16, tag="whhb")
            whh_sb = [whhb[:, G * i:G * (i + 1)] for i in range(4)]
            biasT_sb = [const.tile([8, 128], BF16, name=f"biasT{i}",
                                   tag=f"biasT{i}") for i in range(2)]
            mask_sb = const.tile([8, 128], BF16, tag="mask_sb")
            biasTf_sb = const.tile([16, 128], BF16, tag="biasTf_sb")
            maskf_sb = const.tile([16, 256], BF16, tag="maskf_sb")
            w1T_sb = [const.tile([128, 64], BF16, name=f"w1T{i}", tag=f"w1T{i}")
                      for i in range(4)]
            b1_sb = const.tile([1, 64], BF16, tag="b1_sb")
            w2T_sb = const.tile([64, 2], BF16, tag="w2T_sb")
            b2_sb = const.tile([1, 2], BF16, tag="b2_sb")
            ones_sb = const.tile([1, 16], BF16, tag="ones_sb")

            def load_scan_weights():
                nc.sync.dma_start(
                    wihb[:].rearrange("p (n g) -> p n g", n=2),
                    wihT[0:2].transpose([1, 0, 2]),
                )
                nc.sync.dma_start(
                    whhb[:].rearrange("p (n g) -> p n g", n=4),
                    whhT[0:4].transpose([1, 0, 2]),
                )
                if PS_ONE:
                    nc.sync.dma_start(biasTf_sb[:], biasTf[:])
                    nc.sync.dma_start(maskf_sb[:], maskf[:])
                else:
                    for i in range(2):
                        nc.sync.dma_start(biasT_sb[i][:], biasT[i])
                    nc.sync.dma_start(mask_sb[:], maskb[:])
                for i in range(4):
                    nc.sync.dma_start(w1T_sb[i][:], w1T[i])
                nc.sync.dma_start(b1_sb[:], b1r[:])
                nc.sync.dma_start(w2T_sb[:], w2T[:])
                nc.sync.dma_start(b2_sb[:], b2r[:])
                nc.sync.dma_start(ones_sb[:], ones1[:])

            # ---- persistent activations / state ----
            featsT = [state.tile([128, T, BL], BF16, name=f"featsT{i}",
                                 tag=f"featsT{i}") for i in range(2)]
            c_st = state.tile([128, 4 * BL], F32, tag="c_st")
            nc.vector.memset(c_st[:], 0.0)
            # TAIL_SCAN: per-half interleaved [., c~_0, u_0, c~_1, u_1, ...]
            # buffer (c~ = c/2 at odd cols 1+2j, u at even cols 2+2j).  The
            # c-recurrence c~_j = sigmf_j * c~_j + u_j is ONE DVE
            # tensor_tensor_scan per half: 2-element cells [reset, apply]
            # with d0 = [0, sigmf] (zeros preset at odd cols of the sig
            # tile), d1 = this buffer.
            # ping-pong per step parity: scan(t) reads buf[t%2], writes
            # buf[(t+1)%2]; the o-copy also targets the write buffer
            cil = [[state.tile([128, 132], F32, name=f"cil{h_}{p_}",
                               tag=f"cil{h_}{p_}") for p_ in range(2)]
                   for h_ in range(2)]
            for h_ in range(2):
                for p_ in range(2):
                    nc.vector.memset(cil[h_][p_][:], 0.0)
            # C_FRESH: c lives in a rotating pool instead (see scan loop)
            if skip_conv:
                for i in range(2):
                    nc.vector.memset(featsT[i][:].bitcast(F32), 0.0)

            # ===== Conv (tq-streamed into the scan) + Scan =================
            # conv tiled as 8 time-blocks of 128; block 0 is a short
            # prelude, blocks 1..7 are emitted into the scan loop at a
            # uniform rate so block j is ready before scan step 128*j.
            # The Tile priority scheduler slots the ops into idle engine
            # gaps (PE ~60%, ACT ~50% idle during the scan).
            def conv_piece_thunks(t0, w):
                # conv for output cols [t0, t0+w) over all examples/channels
                thunks = []
                if skip_conv:
                    return thunks
                xts = {}

                def load_x(ex, t0=t0, w=w):
                    lo = t0 - 2
                    pair = []
                    for cc in range(2):
                        t_ = xp.tile([128, w + 4], CDT, name=f"xs{cc}",
                                     tag=f"xs{cc}")
                        s0, s1 = max(lo, 0), min(lo + w + 4, T)
                        if t0 == 0:
                            nc.vector.memset(t_[:, 0:2].bitcast(F32), 0.0)
                        if t0 + w == T:
                            nc.vector.memset(
                                t_[:, w + 2:w + 4].bitcast(F32), 0.0)
                        nc.sync.dma_start(
                            t_[:, s0 - lo:s1 - lo],
                            x_l[ex, 128 * cc:128 * (cc + 1), s0:s1],
                        )
                        pair.append(t_)
                    xts[ex] = pair

                for ex in range(BL):
                    thunks.append(lambda ex=ex: load_x(ex))
                    for co in range(2):
                        cst = {}

                        def mk_mm(ex, co, cc, k, first, w=w, cst=cst):
                            def mm():
                                if first:
                                    cst["ps"] = cpsp.tile([128, w], F32,
                                                          tag="cps",
                                                          name="cps",
                                                          padded_shape=[128, 512])
                                nc.tensor.matmul(
                                    cst["ps"][:],
                                    cw_sb[k * 2 + cc][:, 128 * co:128 * (co + 1)],
                                    xts[ex][cc][:, k:k + w],
                                    start=first,
                                    stop=(cc == 1 and k == 4),
                                )
                            return mm

                        first = True
                        for cc in range(2):
                            for k in range(5):
                                thunks.append(mk_mm(ex, co, cc, k, first))
                                first = False

                        def evict(ex=ex, co=co, t0=t0, w=w, cst=cst):
                            if EVICT_DVE:
                                # bn scale folded into conv weights host-side
                                nc.vector.tensor_scalar(
                                    featsT[co][:, t0:t0 + w, ex],
                                    cst["ps"][:], bn_sb[co][:, 1:2], 0.0,
                                    OP.add, OP.max,
                                )
                            else:
                                nc.scalar.activation(
                                    featsT[co][:, t0:t0 + w, ex],
                                    cst["ps"][:], AF.Relu,
                                    bias=bn_sb[co][:, 1:2],
                                    scale=bn_sb[co][:, 0:1],
                                )
                        thunks.append(evict)
                return thunks

            # ================= Scan (transposed layout) ====================
            with (
                tc.tile_pool(name="xp", bufs=6) as xp,
                tc.tile_pool(name="cps", bufs=CPS_BUFS, space="PSUM") as cpsp,
                tc.tile_pool(name="hTp", bufs=HT_BUFS) as hTp,
                tc.tile_pool(name="sig", bufs=SIG_BUFS) as sigp,
                tc.tile_pool(name="sml", bufs=SIG_BUFS) as smlp,
                tc.tile_pool(name="psA", bufs=PS_BUFS, space="PSUM") as psAp,
                tc.tile_pool(name="psB", bufs=PS_BUFS, space="PSUM") as psBp,
            ):
                # conv prelude: only the first PRE0 cols must precede
                # step 0; the rest of block 0 streams into the first steps
                # with tight deadlines, blocks 1..7 at a uniform rate.
                # prelude piece with the x loads batched into one wide
                # DMA per cc (32 separate dma_starts would serialize ~18us
                # of SP sequencer time before the first conv matmul)
                if not skip_conv:
                    wp = PRE0 + 4
                    xb = [xp.tile([128, BL * wp], CDT, name=f"xb{cc}",
                                  tag=f"xb{cc}") for cc in range(2)]
                    for cc in range(2):
                        xv = xb[cc][:].rearrange("p (e w) -> p e w", e=BL)
                        nc.vector.memset(xv[:, :, 0:2].bitcast(F32), 0.0)
                        nc.sync.dma_start(
                            xv[:, :, 2:wp],
                            x_l[0:BL, 128 * cc:128 * (cc + 1),
                                0:PRE0 + 2].transpose([1, 0, 2]),
                        )
                    for ex in range(BL):
                        for co in range(2):
                            pps = cpsp.tile([128, PRE0], F32, tag="cps",
                                            name="cps",
                                            padded_shape=[128, 128])
                            first = True
                            for cc in range(2):
                                for k in range(5):
                                    nc.tensor.matmul(
                                        pps[:],
                                        cw_sb[k * 2 + cc][:, 128 * co:128 * (co + 1)],
                                        xb[cc][:, ex * wp + k:ex * wp + k + PRE0],
                                        start=first,
                                        stop=(cc == 1 and k == 4),
                                    )
                                    first = False
                            if EVICT_DVE:
                                nc.vector.tensor_scalar(
                                    featsT[co][:, 0:PRE0, ex],
                                    pps[:], bn_sb[co][:, 1:2], 0.0,
                                    OP.add, OP.max,
                                )
                            else:
                                nc.scalar.activation(
                                    featsT[co][:, 0:PRE0, ex],
                                    pps[:], AF.Relu,
                                    bias=bn_sb[co][:, 1:2],
                                    scale=bn_sb[co][:, 0:1],
                                )
                load_scan_weights()
                early_q = []
                for t0, w0 in ((PRE0, PRE0), (2 * PRE0, 2 * PRE0)):
                    early_q += conv_piece_thunks(t0, w0)
                early_dl = 2 * PRE0 - 4
                early_emitted = 0
                # wider mid pieces halve the ACT eviction count; each
                # piece must fully evict before the scan consumes its first
                # column, so each gets its own deadline window.
                conv_segs = []
                prev = 0
                for t0, w in ((128, 256), (384, 256), (640, 384)):
                    s1 = max(1, min(nT, t0 - 6))
                    conv_segs.append(
                        [conv_piece_thunks(t0, w), prev, s1, 0])
                    prev = s1

                hT = hTp.tile([128, 4 * BL], BF16, tag="hT", name="hT")
                nc.vector.memset(hT[:].bitcast(F32), 0.0)
                if C_FRESH:
                    c_cur = hTp.tile([128, 4 * BL], F32, tag="cT", name="cT")
                    nc.vector.memset(c_cur[:], 0.0)
                else:
                    c_cur = c_st
                if TAIL_SCAN and PS_ONE:
                    # preset the sig-pool buffers' odd columns to zero once;
                    # in-loop writers only touch even columns.
                    for _b in range(SIG_BUFS):
                        s0 = sigp.tile([128, 512], F32, tag="sig", name="sig")
                        sv0 = s0[:].rearrange("p (j t) -> p j t", t=2)
                        nc.vector.memset(sv0[:, :, 1], 0.0)
                elif TAIL_SCAN:
                    for _b in range(SIG_BUFS):
                        for hf in range(2):
                            s0 = sigp.tile([128, 256], F32, tag=f"sig{hf}",
                                           name=f"sig{hf}")
                            sv0 = s0[:].rearrange("p (j t) -> p j t", t=2)
                            nc.vector.memset(sv0[:, :, 1], 0.0)

                CH = (A_CH, B_CH)             # chunks per half
                KS = ((0, 1), (2, 3))         # hT k-slices produced per half
                def open_step(t):
                    # allocate this step's gate psums, open the accumulation
                    # groups with the bias matmuls, and emit the input
                    # projection.  Called one step ahead so these (dependency-
                    # free) matmuls sit ahead of the waiting whh matmuls in
                    # PE's in-order queue and fill its idle time.
                    L = (t % 2) if ALT_LEAD else 0
                    R = 1 - L
                    pshape = [128, 512] if PSUM_PAD else None
                    if PS_ONE:
                        pt = psAp.tile([128, 256], F32, tag="ps", name="ps",
                                       padded_shape=pshape)
                        ps = [pt, pt]
                        nc.tensor.matmul(pt[:], biasTf_sb[:], maskf_sb[:],
                                         start=True, stop=False)
                    else:
                        ps = [None, None]
                        ps[L] = (psAp if L == 0 else psBp).tile(
                            [128, 128], F32, tag=f"ps{L}", name=f"ps{L}",
                            padded_shape=pshape)
                        ps[R] = (psAp if R == 0 else psBp).tile(
                            [128, 128], F32, tag=f"ps{R}", name=f"ps{R}",
                            padded_shape=pshape)
                        nc.tensor.matmul(ps[L][:], biasT_sb[L][:], mask_sb[:],
                                         start=True, stop=False)
                        nc.tensor.matmul(ps[R][:], biasT_sb[R][:], mask_sb[:],
                                         start=True, stop=False)
                    for m in CH[L] + CH[R]:
                        p_, c0 = _mm_dest(ps[0], ps[1], m)
                        for cc in range(2):
                            nc.tensor.matmul(
                                p_[:, c0:c0 + BL],
                                wih_sb[cc][:, 128 * m:128 * (m + 1)],
                                featsT[cc][:, t, :],
                                start=False, stop=False,
                            )
                    return ps

                ps_next = open_step(0)
                for t in range(nT):
                    # stream conv emission at a uniform rate
                    etarget = min(len(early_q),
                                  int(len(early_q) * (t + 1) / early_dl))
                    while early_emitted < etarget:
                        early_q[early_emitted]()
                        early_emitted += 1
                    for seg in conv_segs:
                        th, s0, s1, done = seg
                        if t < s0 or done >= len(th):
                            continue
                        tgt = min(len(th),
                                  int(len(th) * (t + 1 - s0) / (s1 - s0)))
                        while seg[3] < tgt:
                            th[seg[3]]()
                            seg[3] += 1
                    # lead half L: its tail (and thus hT slices) finish early;
                    # alternate so the late half of step t leads step t+1.
                    L = (t % 2) if ALT_LEAD else 0
                    R = 1 - L
                    ps = ps_next
                    if t + 1 < nT:
                        ps_next = open_step(t + 1)
                    # recurrent term.  K[R] slices were produced by last
                    # step's lead tail (early) - emit them first; the final
                    # k group is gated by last step's trailing hh.  Within
                    # it, close the lead half's psum first.
                    for k in KS[R]:
                        for m in CH[L] + CH[R]:
                            p_, c0 = _mm_dest(ps[0], ps[1], m)
                            nc.tensor.matmul(
                                p_[:, c0:c0 + BL],
                                whh_sb[k][:, 128 * m:128 * (m + 1)],
                                hT[:, BL * k:BL * (k + 1)],
                                start=False, stop=False,
                            )
                    i_stop = None
                    for chunks in (CH[L], CH[R]):
                        for k in KS[L]:
                            for m in chunks:
                                p_, c0 = _mm_dest(ps[0], ps[1], m)
                                last = k == KS[L][-1] and m == chunks[-1]
                                if PS_ONE:
                                    stop_ = last and chunks is CH[R]
                                else:
                                    stop_ = last
                                i_mm = nc.tensor.matmul(
                                    p_[:, c0:c0 + BL],
                                    whh_sb[k][:, 128 * m:128 * (m + 1)],
                                    hT[:, BL * k:BL * (k + 1)],
                                    start=False, stop=stop_,
                                )
                                if stop_:
                                    i_stop = i_mm

                    hT_new = hTp.tile([128, 4 * BL], BF16, tag="hT", name="hT")
                    if C_FRESH:
                        c_new = hTp.tile([128, 4 * BL], F32, tag="cT",
                                         name="cT")
                    else:
                        c_new = c_cur
                    if TAIL_SCAN and PS_ONE:
                        # one sig tile for both halves: half hf at cols
                        # [256*hf, 256*hf+256), gates at even cols (zeros
                        # preset at odd cols).
                        sg = sigp.tile([128, 512], F32, tag="sig", name="sig")
                        sgv = sg[:].rearrange("p (h j t) -> p h j t",
                                              h=2, t=2)
                        i_sL = nc.scalar.activation(
                            sgv[:, L, 0:96, 0], ps[L][:, 128 * L:128 * L + 96],
                            AF.Sigmoid)
                        i_sR = nc.scalar.activation(
                            sgv[:, R, 0:96, 0], ps[R][:, 128 * R:128 * R + 96],
                            AF.Sigmoid)
                        psv = ps[0][:].rearrange("p (h c) -> p h c", h=2)
                        i_so = nc.scalar.activation(
                            sgv[:, :, 96:128, 0], psv[:, :, 96:128],
                            AF.Sigmoid)
                        # sub-range gating only (same PE-write/ACT-read bank
                        # overlap the 2-bank baseline already runs with on
                        # HW).  SIG_STOPDEP restores strict full-group
                        # gating if needed.
                        if SIG_STOPDEP and i_stop is not None:
                            i_sL.ins.add_dependency(
                                i_stop.ins.name,
                                mybir.DependencyInfo(sync=True, no_sync=False))
                            i_sR.ins.add_dependency(
                                i_sL.ins.name,
                                mybir.DependencyInfo(sync=False, no_sync=True))
                            i_so.ins.add_dependency(
                                i_sR.ins.name,
                                mybir.DependencyInfo(sync=False, no_sync=True))
                        sigs = {L: None, R: None}
                        for hf in (L, R):
                            sv = sgv[:, hf]
                            ci_ = cil[hf]
                            cv = ci_[:].rearrange("p (j t) -> p j t", t=2)
                            i_u = nc.vector.scalar_tensor_tensor(
                                cv[:, 1:33, 0], sv[:, 64:96, 0], -0.5,
                                sv[:, 0:32, 0], OP.add, OP.mult,
                            )
                            i_c = nc.vector.tensor_tensor_scan(
                                ci_[:, 0:64], sg[:, 256 * hf + 63:256 * hf + 127],
                                ci_[:, 1:65], 0.0, OP.mult, OP.add,
                            )
                            _demote(i_c, i_u)
                            sc = smlp.tile([128, 32], F32, tag=f"sc{hf}",
                                           name=f"sc{hf}")
                            nc.scalar.activation(sc[:], cv[:, 0:32, 1],
                                                 AF.Sigmoid, scale=4.0)
                            if HH_SPLIT:
                                for q in range(2):
                                    Sq = slice(32 * hf + 16 * q,
                                               32 * hf + 16 * (q + 1))
                                    nc.vector.scalar_tensor_tensor(
                                        hT_new[:, Sq],
                                        sc[:, 16 * q:16 * (q + 1)], -0.5,
                                        sv[:, 96 + 16 * q:96 + 16 * (q + 1), 0],
                                        OP.add, OP.mult,
                                    )
                            else:
                                nc.vector.scalar_tensor_tensor(
                                    hT_new[:, 32 * hf:32 * (hf + 1)], sc[:],
                                    -0.5, sv[:, 96:128, 0], OP.add, OP.mult,
                                )
                        hT = hT_new
                        continue
                    if TAIL_SCAN:
                        sigs = {}
                        # phase 1: ifg sigmoids (ACT in-order; lead first).
                        # sig layout: gates at even cols (zeros preset at
                        # odd cols).
                        for hf in (L, R):
                            s_ = sigp.tile([128, 256], F32, tag=f"sig{hf}",
                                           name=f"sig{hf}")
                            sv = s_[:].rearrange("p (j t) -> p j t", t=2)
                            nc.scalar.activation(sv[:, 0:96, 0],
                                                 ps[hf][:, 0:96],
                                                 AF.Sigmoid)
                            if not SCO_FUSE:
                                nc.scalar.activation(sv[:, 96:128, 0],
                                                     ps[hf][:, 96:128],
                                                     AF.Sigmoid)
                            sigs[hf] = s_
                        if SCO_FUSE:
                            # o-gates (/4) -> odd cols 67..129 of the WRITE
                            # buffer; slots into DVE's idle window right
                            # after the psum stop, so one ACT sigma(4x)
                            # later yields both sigma(2c) and sigma(o)
                            for hf in (L, R):
                                cvh = cil[hf][(t + 1) % 2][:].rearrange(
                                    "p (j t2) -> p j t2", t2=2)
                                nc.vector.tensor_scalar_mul(
                                    cvh[:, 33:65, 1], ps[hf][:, 96:128], 0.25)
                        # phase 2: per-half u + c-scan + sigma(4x) + h
                        for hf in (L, R):
                            s_ = sigs[hf]
                            sv = s_[:].rearrange("p (j t2) -> p j t2", t2=2)
                            ca = cil[hf][t % 2]
                            cb = cil[hf][(t + 1) % 2]
                            cv_a = ca[:].rearrange("p (j t2) -> p j t2", t2=2)
                            cv = cb[:].rearrange("p (j t2) -> p j t2", t2=2)
                            # u_j -> even cols 2+2j of the read buffer
                            i_u = nc.vector.scalar_tensor_tensor(
                                cv_a[:, 1:33, 0], sv[:, 64:96, 0], -0.5,
                                sv[:, 0:32, 0], OP.add, OP.mult,
                            )
                            # c~_j = sigf_j * c~_j + u_j  (one scan op;
                            # d0 = sig cols 63..126 = [0, f_0, 0, f_1, ...])
                            i_c = nc.vector.tensor_tensor_scan(
                                cb[:, 0:64], s_[:, 63:127], ca[:, 1:65],
                                0.0, OP.mult, OP.add,
                            )
                            _demote(i_c, i_u)
                            if SCO_FUSE:
                                sco = smlp.tile([128, 65], F32,
                                                tag=f"sc{hf}", name=f"sc{hf}")
                                nc.scalar.activation(sco[:], cv[:, 0:65, 1],
                                                     AF.Sigmoid, scale=4.0)
                                if HH_SPLIT:
                                    for q in range(2):
                                        Sq = slice(32 * hf + 16 * q,
                                                   32 * hf + 16 * (q + 1))
                                        nc.vector.scalar_tensor_tensor(
                                            hT_new[:, Sq],
                                            sco[:, 16 * q:16 * (q + 1)], -0.5,
                                            sco[:, 33 + 16 * q:49 + 16 * q],
                                            OP.add, OP.mult,
                                        )
                                else:
                                    nc.vector.scalar_tensor_tensor(
                                        hT_new[:, 32 * hf:32 * (hf + 1)],
                                        sco[:, 0:32], -0.5, sco[:, 33:65],
                                        OP.add, OP.mult,
                                    )
                                continue
                            sc = smlp.tile([128, 32], F32, tag=f"sc{hf}",
                                           name=f"sc{hf}")
                            nc.scalar.activation(sc[:], cv[:, 0:32, 1],
                                                 AF.Sigmoid, scale=4.0)
                            if HH_SPLIT:
                                for q in range(2):
                                    Sq = slice(32 * hf + 16 * q,
                                               32 * hf + 16 * (q + 1))
                                    nc.vector.scalar_tensor_tensor(
                                        hT_new[:, Sq],
                                        sc[:, 16 * q:16 * (q + 1)], -0.5,
                                        sv[:, 96 + 16 * q:96 + 16 * (q + 1), 0],
                                        OP.add, OP.mult,
                                    )
                            else:
                                nc.vector.scalar_tensor_tensor(
                                    hT_new[:, 32 * hf:32 * (hf + 1)], sc[:],
                                    -0.5, sv[:, 96:128, 0], OP.add, OP.mult,
                                )
                        hT = hT_new
                        if C_FRESH:
                            c_cur = c_new
                        continue
                    sigs = {}
                    # sigmoids first (ACT is in-order; lead half first)
                    for hf in (L, R):
                        s_ = sigp.tile([128, 128], F32, tag=f"sig{hf}",
                                       name=f"sig{hf}")
                        # i,f,g first (gates the c chain), o later
                        nc.scalar.activation(s_[:, 0:96], ps[hf][:, 0:96],
                                             AF.Sigmoid)
                        nc.scalar.activation(s_[:, 96:128],
                                             ps[hf][:, 96:128], AF.Sigmoid)
                        sigs[hf] = s_
                        S = slice(32 * hf, 32 * (hf + 1))
                        if TAIL_DVE:
                            # v, u, c engine-chained on DVE; c's RAW deps on
                            # u and v are enforced by DVE program order, so
                            # the sems are demoted to nosync edges.
                            v = smlp.tile([128, 32], F32, tag=f"v{hf}",
                                          name=f"v{hf}")
                            i_v = nc.vector.tensor_mul(
                                v[:], s_[:, 32:64], c_cur[:, S])
                            u = smlp.tile([128, 32], F32, tag=f"u{hf}",
                                          name=f"u{hf}")
                            i_u = nc.vector.scalar_tensor_tensor(
                                u[:], s_[:, 64:96], -0.5, s_[:, 0:32],
                                OP.add, OP.mult,
                            )
                            i_c = nc.vector.scalar_tensor_tensor(
                                c_new[:, S], u[:], 2.0, v[:],
                                OP.mult, OP.add,
                            )
                            _demote(i_c, i_u, i_v)
                        else:
                            # c update for this half (only TensorTensor is
                            # legal on Pool, so at most the v-multiply can
                            # be offloaded there)
                            v_pool = (POOL_V == 2
                                      or (POOL_V == 1 and hf == R)
                                      or (POOL_V == 3 and hf == L))
                            u = smlp.tile([128, 32], F32, tag=f"u{hf}",
                                          name=f"u{hf}")
                            nc.vector.scalar_tensor_tensor(
                                u[:], s_[:, 64:96], -0.5, s_[:, 0:32],
                                OP.add, OP.mult,
                            )
                            v = smlp.tile([128, 32], F32, tag=f"v{hf}",
                                          name=f"v{hf}")
                            veng = nc.gpsimd if v_pool else nc.vector
                            veng.tensor_mul(v[:], s_[:, 32:64], c_cur[:, S])
                            nc.vector.scalar_tensor_tensor(
                                c_new[:, S], u[:], 2.0, v[:],
                                OP.mult, OP.add,
                            )
                    for hf in (L, R):
                        S = slice(32 * hf, 32 * (hf + 1))
                        sc = smlp.tile([128, 32], F32, tag=f"sc{hf}",
                                       name=f"sc{hf}")
                        nc.scalar.activation(sc[:], c_new[:, S], AF.Sigmoid,
                                             scale=2.0)
                        if HH_SPLIT:
                            for q in range(2):
                                Sq = slice(32 * hf + 16 * q,
                                           32 * hf + 16 * (q + 1))
                                nc.vector.scalar_tensor_tensor(
                                    hT_new[:, Sq], sc[:, 16 * q:16 * (q + 1)],
                                    -0.5, sigs[hf][:, 96 + 16 * q:96 + 16 * (q + 1)],
                                    OP.add, OP.mult,
                                )
                        else:
                            nc.vector.scalar_tensor_tensor(
                                hT_new[:, S], sc[:], -0.5, sigs[hf][:, 96:128],
                                OP.add, OP.mult,
                            )
                    hT = hT_new
                    if C_FRESH:
                        c_cur = c_new
                while early_emitted < len(early_q):
                    early_q[early_emitted]()
                    early_emitted += 1
                for seg in conv_segs:
                    th = seg[0]
                    while seg[3] < len(th):
                        th[seg[3]]()
                        seg[3] += 1

            # ================= Head ========================================
            with (
                tc.tile_pool(name="hd", bufs=1) as hd,
                tc.tile_pool(name="hps", bufs=1, space="PSUM") as hpsp,
            ):
                hps = hpsp.tile([64, BL], F32, tag="hps")
                nc.tensor.matmul(hps[:], b1_sb[:], ones_sb[:],
                                 start=True, stop=False)
                for k in range(4):
                    nc.tensor.matmul(
                        hps[:], w1T_sb[k][:], hT[:, BL * k:BL * (k + 1)],
                        start=False, stop=(k == 3),
                    )
                hid = hd.tile([64, BL], BF16, tag="hid")
                nc.scalar.activation(hid[:], hps[:], AF.Relu)
                lps = hpsp.tile([2, BL], F32, tag="lps")
                nc.tensor.matmul(lps[:], b2_sb[:], ones_sb[:],
                                 start=True, stop=False)
                nc.tensor.matmul(lps[:], w2T_sb[:], hid[:],
                                 start=False, stop=True)
                outt = hd.tile([2, BL], F32, tag="outt")
                nc.vector.tensor_copy(outt[:], lps[:])
                nc.sync.dma_start(out[:], outt[:])

    nc.compile()
    return nc


def _prep(inputs):
    x = np.asarray(inputs["x"], np.float32)
    conv_w = np.asarray(inputs["conv_w"], np.float32)
    bn_gamma = np.asarray(inputs["bn_gamma"], np.float32)
    bn_beta = np.asarray(inputs["bn_beta"], np.float32)
    w_ih = np.asarray(inputs["w_ih"], np.float32)
    w_hh = np.asarray(inputs["w_hh"], np.float32)
    b_ih = np.asarray(inputs["b_ih"], np.float32)
    b_hh = np.asarray(inputs["b_hh"], np.float32)
    w1 = np.asarray(inputs["w1"], np.float32)
    b1 = np.asarray(inputs["b1"], np.float32)
    w2 = np.asarray(inputs["w2"], np.float32)
    b2 = np.asarray(inputs["b2"], np.float32)
    bf = ml_dtypes.bfloat16

    # ---- BN batch statistics (host, exact) ----
    xp_ = np.pad(x, ((0, 0), (0, 0), (2, 2)))
    Xt = np.ascontiguousarray(xp_.transpose(1, 0, 2))  # [C, B, T+4]
    acc = np.zeros((C, B, T), np.float32)
    for k in range(5):
        acc += np.tensordot(conv_w[:, :, k], Xt[:, :, k:k + T], axes=(1, 0))
    mean = acc.mean(axis=(1, 2), dtype=np.float64)
    var = (acc.astype(np.float64) ** 2).mean(axis=(1, 2)) - mean ** 2
    bn_a = (bn_gamma.astype(np.float64) / np.sqrt(var + EPS))
    bn_b = bn_beta.astype(np.float64) - mean * bn_a
    bn_ab = np.stack([bn_a, bn_b], axis=1).astype(np.float32)  # [C, 2]

    # ---- gate permutation: [i | f | o | g] with g rows scaled x2 ----
    perm = np.r_[0:512, 512:1024, 1536:2048, 1024:1536]
    rs = np.ones((G, 1), np.float32)
    rs[1536:2048] = 2.0

    w_ih_p = w_ih[perm] * rs                       # [G, C]
    w_hh_p = w_hh[perm] * rs * 2.0                 # [G, H]
    bias_p = ((b_ih + b_hh)[perm] * rs[:, 0])      # [G]

    wihT = np.ascontiguousarray(w_ih_p.T.reshape(2, 128, G)).astype(bf)
    whhT = np.ascontiguousarray(w_hh_p.T.reshape(4, 128, G)).astype(bf)

    bias4 = bias_p.reshape(4, 4, 128)[[0, 1, 3, 2]]  # col order [i,f,g,o]
    biasT = np.stack([
        bias4[:, 0:2, :].reshape(8, 128),
        bias4[:, 2:4, :].reshape(8, 128),
    ]).astype(bf)                                   # [half, j=col*2+s2, gp]
    maskb = np.zeros((8, 128), np.float32)
    for j in range(8):
        maskb[j, 16 * j:16 * (j + 1)] = 1.0
    maskb = maskb.astype(bf)
    # single-tile variants: j = 8*half + GT2COL[gt]*2 + s2 -> 16-col block
    biasTf = np.zeros((16, 128), np.float32)
    for j in range(16):
        h_, r = j // 8, j % 8
        gt = GT2COL.index(r // 2)
        m = gt * 4 + 2 * h_ + (r % 2)
        biasTf[j] = bias_p[128 * m:128 * (m + 1)]
    biasTf = biasTf.astype(bf)
    maskf = np.zeros((16, 256), np.float32)
    for j in range(16):
        maskf[j, 16 * j:16 * (j + 1)] = 1.0
    maskf = maskf.astype(bf)

    cdt = bf if CONV_BF16 else np.float32
    cw = np.zeros((10, 128, 256), np.float32)
    for k in range(5):
        for cc in range(2):
            cw[k * 2 + cc] = conv_w[:, 128 * cc:128 * (cc + 1), k].T
    if EVICT_DVE:
        # fold the BN scale into the conv weights (per output channel =
        # last axis of cw); eviction then only needs (+bias, relu)
        cw = cw * bn_a.astype(np.float32)[None, None, :]
    cw = cw.astype(cdt)

    w1T = np.ascontiguousarray((2.0 * w1).T.reshape(4, 128, 64)).astype(bf)
    w2T = np.ascontiguousarray(w2.T).astype(bf)

    common = dict(
        cw=cw,
        bn_ab=bn_ab,
        wihT=wihT,
        whhT=whhT,
        biasT=biasT,
        maskb=maskb,
        biasTf=biasTf,
        maskf=maskf,
        w1T=w1T,
        b1r=b1.reshape(1, 64).astype(bf),
        w2T=w2T,
        b2r=b2.reshape(1, 2).astype(bf),
        ones1=np.ones((1, 16), bf),
    )
    in_maps = []
    x_c = x.astype(cdt)
    for core in range(NCORES):
        m = dict(common)
        m["x_l"] = np.ascontiguousarray(x_c[BL * core:BL * (core + 1)])
        in_maps.append(m)
    return in_maps


def kernel(**inputs) -> np.ndarray:
    if "nc" not in _cache:
        _cache["nc"] = _build()
    nc = _cache["nc"]
    in_maps = _prep(inputs)
    res = run_bass_kernel_spmd(nc, in_maps, list(range(NCORES)))
    _cache["last_res"] = res
    return np.concatenate([res.results[c]["out"].T for c in range(NCORES)],
                          axis=0).astype(np.float32)



# revision 44
# speedup vs baseline: 1.0012x; 1.0012x over previous
"""CNN-LSTM decoder Trainium2 kernel (8 NeuronCores, data-parallel over batch).

Model (see reference): Conv1d(256->256,k=5,p=2) -> BatchNorm1d(train stats) ->
ReLU -> LSTM(256->512, T=1024) -> Linear(512->64) -> ReLU -> Linear(64->2).

Distribution: batch 128 split 16/core across 8 cores.  Per core:

  Conv:    time-block tiled ([128, w+4] x slices, N=w bf16 matmuls,
           BN+ReLU fused into the ACT eviction straight into a persistent
           SBUF buffer featsT[cc][ci, t, b] in bf16).  Only the first 32
           output cols are computed before the scan starts; the rest is
           emitted into the scan loop against per-piece consumption
           deadlines and hidden in engine idle gaps by the Tile priority
           scheduler.  BN batch stats are computed host-side.

  Scan:    fully transposed layout.  Gates live as gatesT [128 gate-dims,
           16 batch] PSUM tiles, two halves (H-slices 0-1 / 2-3, one PSUM
           bank each); psum cols = [i|f|g|o]*32 + s2*16 + b.  Per step:
             - 2 bias matmuls (K=8 mask trick) open each half's accum group
             - 32 input-projection matmuls (wihT chunks @ featsT[:, t, :]),
               emitted one step ahead so they fill PE idle time
             - 64 recurrent matmuls (whhT chunks @ hT k-slices), N=16 bf16
           Tail per half (TAIL_SCAN): one sigmoid over i/f/g writing to
           EVEN columns of the sig tile (odd columns preset to zero; g rows
           pre-scaled x2 so tanh(x) = 2*sigmoid(2x)-1), sigmoid over o,
           then u = (sig2g-.5)*sigi into the even cols of an interleaved
           [., c~, u, c~, u, ...] buffer and the whole c-update
           c~_j = sigf_j*c~_j + u_j as ONE DVE tensor_tensor_scan
           (2-element cells [reset, apply]; d0 = [0|sigf] interleaved via
           the preset zeros, ping-pong buffers per step parity).  c~ = c/2,
           so sigma(2c) = sigma(4c~).  h = (sig(4c~)-.5)*sigo is written
           as h/2 in bf16, the exact moving operand of the next step's
           recurrent matmuls.  Same-engine RAW deps in the tail are demoted
           to nosync edges (_demote): in-order engines make the semaphores
           redundant, removing their latency from the recurrence chain.
           The half whose tail finished late leads the next step
           (alternating), keeping both ACT chains near-saturated.  The
           cycle is bound by the dense ACT sequence (2 sigma96 + 2 sigma_o
           + 2 sigma(4c~)) plus the psum-stop coupling.

  Head:    transposed too: hid [64, 16] = relu(w1 @ h + b1); out [2, 16].

All host-side weight massaging (permutation, scaling, transposes, BN stats)
happens in kernel() below; the device kernel is compiled once per process.
"""

import sys

sys.path.insert(0, "/opt/trn_rl_repo")

import ml_dtypes
import numpy as np

import concourse.bass as bass
import concourse.tile as tile
from concourse import bacc, mybir
from concourse.bass_utils import run_bass_kernel_spmd

F32 = mybir.dt.float32
F32R = mybir.dt.float32r
BF16 = mybir.dt.bfloat16
AF = mybir.ActivationFunctionType
OP = mybir.AluOpType

B, C, T, H = 128, 256, 1024, 512
G = 4 * H  # 2048
NCORES = 8
BL = B // NCORES  # 16 examples per core
EPS = 1e-5

# gate chunk m = gt*4 + s (gt in [i,f,o,g], s = H-slice 0..3)
# psum half = s // 2; col within half = gt*32 + (s % 2)*16 + b
# within each half, o-gate chunks (gt==2 in perm order) go last so the
# ifg sigmoid is not gated on the o-column matmuls
A_CHUNKS = ([m for m in range(16) if (m % 4) < 2 and m // 4 != 2]
            + [m for m in range(16) if (m % 4) < 2 and m // 4 == 2])
B_CHUNKS = ([m for m in range(16) if (m % 4) >= 2 and m // 4 != 2]
            + [m for m in range(16) if (m % 4) >= 2 and m // 4 == 2])

PSUM_PAD = True
ALT_LEAD = True
HH_SPLIT = True
C_FRESH = False
POOL_V = 1
KORD = 0
SGROUP = 0
EVICT_DVE = False
CPS_BUFS = 3
DL_MARGIN = 0
HT_BUFS = 2
SIG_BUFS = 2
PS_BUFS = 2
PRE0 = 32
SIG_SPLIT = 2
CONV_STREAM = True
CONV_BF16 = True
DEMOTE = True   # demote same-engine data deps to nosync (engine order
                # guarantees RAW on HW: in-order engines drain the pipe
                # between instructions)
TAIL_DVE = True   # (pre-scan fallback) v on DVE ahead of u/c, engine-chained
TAIL_SCAN = True  # c-update as one tensor_tensor_scan over the interleaved
                  # [c~|u] ping-pong buffers (c~ = c/2)
SIG_MERGE = False   # (tried) merge sigma_ifg+sigma_o: breaks phase stagger
PS_ONE = False      # (tried) both halves in one psum bank: re-phases badly
SIG_STOPDEP = False
CONV_SEGS = ((128, 192), (320, 192), (512, 192), (704, 192), (896, 128))
EARLY_DL = 90   # step by which the early (2xPRE0-wide) pieces must land
SCO_FUSE = False    # (tried) fuse sigma_o into sigma(4c~): o-copy lands on
                    # the DVE critical chain

_cache = {}


def _demote(cons, *prods):
    """Demote cons's sync deps on prods to nosync deps.

    Only valid when cons and every prod run on the same engine: the
    scheduler still enforces ordering, and in-order engine execution
    makes the RAW safe without a semaphore."""
    if not DEMOTE:
        return
    ci = cons.ins
    sync = ci.sync_dependency_set_copy()
    for p in prods:
        pi = p.ins
        assert pi.engine == ci.engine, (pi.engine, ci.engine)
        if pi.name in sync:
            ci.remove_dependency(pi.name)
            ci.add_dependency(
                pi.name, mybir.DependencyInfo(sync=False, no_sync=True)
            )


GT2COL = (0, 1, 3, 2)  # perm order [i,f,o,g] -> col blocks [i,f,g,o]


def _mm_dest(psA, psB, m):
    gt, s = m // 4, m % 4
    if PS_ONE:
        c0 = 128 * (s // 2) + GT2COL[gt] * 32 + (s % 2) * 16
        return psA, c0
    ps = psA if s < 2 else psB
    c0 = GT2COL[gt] * 32 + (s % 2) * 16
    return ps, c0


def _build(nT=T, skip_conv=False):
    CDT = BF16 if CONV_BF16 else F32R
    nc = bacc.Bacc("TRN2", target_bir_lowering=False, debug=False,
                   num_devices=NCORES)

    x_l = nc.dram_tensor("x_l", [BL, C, T], CDT, kind="ExternalInput").ap()
    cw = nc.dram_tensor("cw", [10, 128, 256], CDT, kind="ExternalInput").ap()
    bn_ab = nc.dram_tensor("bn_ab", [C, 2], F32, kind="ExternalInput").ap()
    wihT = nc.dram_tensor("wihT", [2, 128, G], BF16, kind="ExternalInput").ap()
    whhT = nc.dram_tensor("whhT", [4, 128, G], BF16, kind="ExternalInput").ap()
    biasT = nc.dram_tensor("biasT", [2, 8, 128], BF16, kind="ExternalInput").ap()
    maskb = nc.dram_tensor("maskb", [8, 128], BF16, kind="ExternalInput").ap()
    biasTf = nc.dram_tensor("biasTf", [16, 128], BF16, kind="ExternalInput").ap()
    maskf = nc.dram_tensor("maskf", [16, 256], BF16, kind="ExternalInput").ap()
    w1T = nc.dram_tensor("w1T", [4, 128, 64], BF16, kind="ExternalInput").ap()
    b1r = nc.dram_tensor("b1r", [1, 64], BF16, kind="ExternalInput").ap()
    w2T = nc.dram_tensor("w2T", [64, 2], BF16, kind="ExternalInput").ap()
    b2r = nc.dram_tensor("b2r", [1, 2], BF16, kind="ExternalInput").ap()
    ones1 = nc.dram_tensor("ones1", [1, 16], BF16, kind="ExternalInput").ap()

    out = nc.dram_tensor("out", [2, BL], F32, kind="ExternalOutput").ap()

    if SCO_FUSE:
        # o-gate chunks first: their psum cols finish early so the DVE
        # o-copies run during the whh block, fully hidden
        A_CH = ([m for m in range(16) if (m % 4) < 2 and m // 4 == 2]
                + [m for m in range(16) if (m % 4) < 2 and m // 4 != 2])
        B_CH = ([m for m in range(16) if (m % 4) >= 2 and m // 4 == 2]
                + [m for m in range(16) if (m % 4) >= 2 and m // 4 != 2])
    else:
        A_CH, B_CH = A_CHUNKS, B_CHUNKS

    with tile.TileContext(nc) as tc:
        with (
            tc.tile_pool(name="const", bufs=1) as const,
            tc.tile_pool(name="state", bufs=1) as state,
        ):
            # ---- persistent constants in SBUF ----
            cwb = const.tile([128, 10 * 256], CDT, tag="cwb")
            cw_sb = [cwb[:, 256 * i:256 * (i + 1)] for i in range(10)]
            nc.sync.dma_start(
                cwb[:].rearrange("p (n c) -> p n c", n=10),
                cw[0:10].transpose([1, 0, 2]),
            )
            bn_sb = [const.tile([128, 2], F32, name=f"bn{i}", tag=f"bn{i}")
                     for i in range(2)]
            for i in range(2):
                nc.sync.dma_start(bn_sb[i][:], bn_ab[128 * i:128 * (i + 1), :])
            # (DMAs for the tensors below are emitted after the conv
            # prelude so the prelude's x slices go first in the DMA queue;
            # they only have to arrive before scan step 0's matmuls.)
            wihb = const.tile([128, 2 * G], BF16, tag="wihb")
            wih_sb = [wihb[:, G * i:G * (i + 1)] for i in range(2)]
            whhb = const.tile([128, 4 * G], BF16, tag="whhb")
            whh_sb = [whhb[:, G * i:G * (i + 1)] for i in range(4)]
            biasT_sb = [const.tile([8, 128], BF16, name=f"biasT{i}",
                                   tag=f"biasT{i}") for i in range(2)]
            mask_sb = const.tile([8, 128], BF16, tag="mask_sb")
            biasTf_sb = const.tile([16, 128], BF16, tag="biasTf_sb")
            maskf_sb = const.tile([16, 256], BF16, tag="maskf_sb")
            w1T_sb = [const.tile([128, 64], BF16, name=f"w1T{i}", tag=f"w1T{i}")
                      for i in range(4)]
            b1_sb = const.tile([1, 64], BF16, tag="b1_sb")
            w2T_sb = const.tile([64, 2], BF16, tag="w2T_sb")
            b2_sb = const.tile([1, 2], BF16, tag="b2_sb")
            ones_sb = const.tile([1, 16], BF16, tag="ones_sb")

            def load_scan_weights():
                nc.sync.dma_start(
                    wihb[:].rearrange("p (n g) -> p n g", n=2),
                    wihT[0:2].transpose([1, 0, 2]),
                )
                nc.sync.dma_start(
                    whhb[:].rearrange("p (n g) -> p n g", n=4),
                    whhT[0:4].transpose([1, 0, 2]),
                )
                if PS_ONE:
                    nc.sync.dma_start(biasTf_sb[:], biasTf[:])
                    nc.sync.dma_start(maskf_sb[:], maskf[:])
                else:
                    for i in range(2):
                        nc.sync.dma_start(biasT_sb[i][:], biasT[i])
                    nc.sync.dma_start(mask_sb[:], maskb[:])
                for i in range(4):
                    nc.sync.dma_start(w1T_sb[i][:], w1T[i])
                nc.sync.dma_start(b1_sb[:], b1r[:])
                nc.sync.dma_start(w2T_sb[:], w2T[:])
                nc.sync.dma_start(b2_sb[:], b2r[:])
                nc.sync.dma_start(ones_sb[:], ones1[:])

            # ---- persistent activations / state ----
            featsT = [state.tile([128, T, BL], BF16, name=f"featsT{i}",
                                 tag=f"featsT{i}") for i in range(2)]
            c_st = state.tile([128, 4 * BL], F32, tag="c_st")
            nc.vector.memset(c_st[:], 0.0)
            # TAIL_SCAN: per-half interleaved [., c~_0, u_0, c~_1, u_1, ...]
            # buffer (c~ = c/2 at odd cols 1+2j, u at even cols 2+2j).  The
            # c-recurrence c~_j = sigmf_j * c~_j + u_j is ONE DVE
            # tensor_tensor_scan per half: 2-element cells [reset, apply]
            # with d0 = [0, sigmf] (zeros preset at odd cols of the sig
            # tile), d1 = this buffer.
            # ping-pong per step parity: scan(t) reads buf[t%2], writes
            # buf[(t+1)%2]; the o-copy also targets the write buffer
            cil = [[state.tile([128, 132], F32, name=f"cil{h_}{p_}",
                               tag=f"cil{h_}{p_}") for p_ in range(2)]
                   for h_ in range(2)]
            for h_ in range(2):
                for p_ in range(2):
                    nc.vector.memset(cil[h_][p_][:], 0.0)
            # C_FRESH: c lives in a rotating pool instead (see scan loop)
            if skip_conv:
                for i in range(2):
                    nc.vector.memset(featsT[i][:].bitcast(F32), 0.0)

            # ===== Conv (tq-streamed into the scan) + Scan =================
            # conv tiled as 8 time-blocks of 128; block 0 is a short
            # prelude, blocks 1..7 are emitted into the scan loop at a
            # uniform rate so block j is ready before scan step 128*j.
            # The Tile priority scheduler slots the ops into idle engine
            # gaps (PE ~60%, ACT ~50% idle during the scan).
            def conv_piece_thunks(t0, w):
                # conv for output cols [t0, t0+w) over all examples/channels
                thunks = []
                if skip_conv:
                    return thunks
                xts = {}

                def load_x(ex, t0=t0, w=w):
                    lo = t0 - 2
                    pair = []
                    for cc in range(2):
                        t_ = xp.tile([128, w + 4], CDT, name=f"xs{cc}",
                                     tag=f"xs{cc}")
                        s0, s1 = max(lo, 0), min(lo + w + 4, T)
                        if t0 == 0:
                            nc.vector.memset(t_[:, 0:2].bitcast(F32), 0.0)
                        if t0 + w == T:
                            nc.vector.memset(
                                t_[:, w + 2:w + 4].bitcast(F32), 0.0)
                        nc.sync.dma_start(
                            t_[:, s0 - lo:s1 - lo],
                            x_l[ex, 128 * cc:128 * (cc + 1), s0:s1],
                        )
                        pair.append(t_)
                    xts[ex] = pair

                for ex in range(BL):
                    thunks.append(lambda ex=ex: load_x(ex))
                    for co in range(2):
                        cst = {}

                        def mk_mm(ex, co, cc, k, first, w=w, cst=cst):
                            def mm():
                                if first:
                                    cst["ps"] = cpsp.tile([128, w], F32,
                                                          tag="cps",
                                                          name="cps",
                                                          padded_shape=[128, 512])
                                nc.tensor.matmul(
                                    cst["ps"][:],
                                    cw_sb[k * 2 + cc][:, 128 * co:128 * (co + 1)],
                                    xts[ex][cc][:, k:k + w],
                                    start=first,
                                    stop=(cc == 1 and k == 4),
                                )
                            return mm

                        first = True
                        for cc in range(2):
                            for k in range(5):
                                thunks.append(mk_mm(ex, co, cc, k, first))
                                first = False

                        def evict(ex=ex, co=co, t0=t0, w=w, cst=cst):
                            if EVICT_DVE:
                                # bn scale folded into conv weights host-side
                                nc.vector.tensor_scalar(
                                    featsT[co][:, t0:t0 + w, ex],
                                    cst["ps"][:], bn_sb[co][:, 1:2], 0.0,
                                    OP.add, OP.max,
                                )
                            else:
                                nc.scalar.activation(
                                    featsT[co][:, t0:t0 + w, ex],
                                    cst["ps"][:], AF.Relu,
                                    bias=bn_sb[co][:, 1:2],
                                    scale=bn_sb[co][:, 0:1],
                                )
                        thunks.append(evict)
                return thunks

            # ================= Scan (transposed layout) ====================
            with (
                tc.tile_pool(name="xp", bufs=6) as xp,
                tc.tile_pool(name="cps", bufs=CPS_BUFS, space="PSUM") as cpsp,
                tc.tile_pool(name="hTp", bufs=HT_BUFS) as hTp,
                tc.tile_pool(name="sig", bufs=SIG_BUFS) as sigp,
                tc.tile_pool(name="sml", bufs=SIG_BUFS) as smlp,
                tc.tile_pool(name="psA", bufs=PS_BUFS, space="PSUM") as psAp,
                tc.tile_pool(name="psB", bufs=PS_BUFS, space="PSUM") as psBp,
            ):
                # conv prelude: only the first PRE0 cols must precede
                # step 0; the rest of block 0 streams into the first steps
                # with tight deadlines, blocks 1..7 at a uniform rate.
                # prelude piece with the x loads batched into one wide
                # DMA per cc (32 separate dma_starts would serialize ~18us
                # of SP sequencer time before the first conv matmul)
                if not skip_conv:
                    wp = PRE0 + 4
                    xb = [xp.tile([128, BL * wp], CDT, name=f"xb{cc}",
                                  tag=f"xb{cc}") for cc in range(2)]
                    for cc in range(2):
                        xv = xb[cc][:].rearrange("p (e w) -> p e w", e=BL)
                        nc.vector.memset(xv[:, :, 0:2].bitcast(F32), 0.0)
                        nc.sync.dma_start(
                            xv[:, :, 2:wp],
                            x_l[0:BL, 128 * cc:128 * (cc + 1),
                                0:PRE0 + 2].transpose([1, 0, 2]),
                        )
                    for ex in range(BL):
                        for co in range(2):
                            pps = cpsp.tile([128, PRE0], F32, tag="cps",
                                            name="cps",
                                            padded_shape=[128, 128])
                            first = True
                            for cc in range(2):
                                for k in range(5):
                                    nc.tensor.matmul(
                                        pps[:],
                                        cw_sb[k * 2 + cc][:, 128 * co:128 * (co + 1)],
                                        xb[cc][:, ex * wp + k:ex * wp + k + PRE0],
                                        start=first,
                                        stop=(cc == 1 and k == 4),
                                    )
                                    first = False
                            if EVICT_DVE:
                                nc.vector.tensor_scalar(
                                    featsT[co][:, 0:PRE0, ex],
                                    pps[:], bn_sb[co][:, 1:2], 0.0,
                                    OP.add, OP.max,
                                )
                            else:
                                nc.scalar.activation(
                                    featsT[co][:, 0:PRE0, ex],
                                    pps[:], AF.Relu,
                                    bias=bn_sb[co][:, 1:2],
                                    scale=bn_sb[co][:, 0:1],
                                )
                load_scan_weights()
                early_q = []
                for t0, w0 in ((PRE0, PRE0), (2 * PRE0, 2 * PRE0)):
                    early_q += conv_piece_thunks(t0, w0)
                early_dl = EARLY_DL
                early_emitted = 0
                # wider mid pieces halve the ACT eviction count; each
                # piece must fully evict before the scan consumes its first
                # column, so each gets its own deadline window.
                conv_segs = []
                prev = 0
                for t0, w in CONV_SEGS:
                    s1 = max(1, min(nT, t0 - 6))
                    conv_segs.append(
                        [conv_piece_thunks(t0, w), prev, s1, 0])
                    prev = s1

                hT = hTp.tile([128, 4 * BL], BF16, tag="hT", name="hT")
                nc.vector.memset(hT[:].bitcast(F32), 0.0)
                if C_FRESH:
                    c_cur = hTp.tile([128, 4 * BL], F32, tag="cT", name="cT")
                    nc.vector.memset(c_cur[:], 0.0)
                else:
                    c_cur = c_st
                if TAIL_SCAN and PS_ONE:
                    # preset the sig-pool buffers' odd columns to zero once;
                    # in-loop writers only touch even columns.
                    for _b in range(SIG_BUFS):
                        s0 = sigp.tile([128, 512], F32, tag="sig", name="sig")
                        sv0 = s0[:].rearrange("p (j t) -> p j t", t=2)
                        nc.vector.memset(sv0[:, :, 1], 0.0)
                elif TAIL_SCAN:
                    for _b in range(SIG_BUFS):
                        for hf in range(2):
                            s0 = sigp.tile([128, 256], F32, tag=f"sig{hf}",
                                           name=f"sig{hf}")
                            sv0 = s0[:].rearrange("p (j t) -> p j t", t=2)
                            nc.vector.memset(sv0[:, :, 1], 0.0)

                CH = (A_CH, B_CH)             # chunks per half
                KS = ((0, 1), (2, 3))         # hT k-slices produced per half
                def open_step(t):
                    # allocate this step's gate psums, open the accumulation
                    # groups with the bias matmuls, and emit the input
                    # projection.  Called one step ahead so these (dependency-
                    # free) matmuls sit ahead of the waiting whh matmuls in
                    # PE's in-order queue and fill its idle time.
                    L = (t % 2) if ALT_LEAD else 0
                    R = 1 - L
                    pshape = [128, 512] if PSUM_PAD else None
                    if PS_ONE:
                        pt = psAp.tile([128, 256], F32, tag="ps", name="ps",
                                       padded_shape=pshape)
                        ps = [pt, pt]
                        nc.tensor.matmul(pt[:], biasTf_sb[:], maskf_sb[:],
                                         start=True, stop=False)
                    else:
                        ps = [None, None]
                        ps[L] = (psAp if L == 0 else psBp).tile(
                            [128, 128], F32, tag=f"ps{L}", name=f"ps{L}",
                            padded_shape=pshape)
                        ps[R] = (psAp if R == 0 else psBp).tile(
                            [128, 128], F32, tag=f"ps{R}", name=f"ps{R}",
                            padded_shape=pshape)
                        nc.tensor.matmul(ps[L][:], biasT_sb[L][:], mask_sb[:],
                                         start=True, stop=False)
                        nc.tensor.matmul(ps[R][:], biasT_sb[R][:], mask_sb[:],
                                         start=True, stop=False)
                    for m in CH[L] + CH[R]:
                        p_, c0 = _mm_dest(ps[0], ps[1], m)
                        for cc in range(2):
                            nc.tensor.matmul(
                                p_[:, c0:c0 + BL],
                                wih_sb[cc][:, 128 * m:128 * (m + 1)],
                                featsT[cc][:, t, :],
                                start=False, stop=False,
                            )
                    return ps

                ps_next = open_step(0)
                for t in range(nT):
                    # stream conv emission at a uniform rate
                    etarget = min(len(early_q),
                                  int(len(early_q) * (t + 1) / early_dl))
                    while early_emitted < etarget:
                        early_q[early_emitted]()
                        early_emitted += 1
                    for seg in conv_segs:
                        th, s0, s1, done = seg
                        if t < s0 or done >= len(th):
                            continue
                        tgt = min(len(th),
                                  int(len(th) * (t + 1 - s0) / (s1 - s0)))
                        while seg[3] < tgt:
                            th[seg[3]]()
                            seg[3] += 1
                    # lead half L: its tail (and thus hT slices) finish early;
                    # alternate so the late half of step t leads step t+1.
                    L = (t % 2) if ALT_LEAD else 0
                    R = 1 - L
                    ps = ps_next
                    if t + 1 < nT:
                        ps_next = open_step(t + 1)
                    # recurrent term.  K[R] slices were produced by last
                    # step's lead tail (early) - emit them first; the final
                    # k group is gated by last step's trailing hh.  Within
                    # it, close the lead half's psum first.
                    for k in KS[R]:
                        for m in CH[L] + CH[R]:
                            p_, c0 = _mm_dest(ps[0], ps[1], m)
                            nc.tensor.matmul(
                                p_[:, c0:c0 + BL],
                                whh_sb[k][:, 128 * m:128 * (m + 1)],
                                hT[:, BL * k:BL * (k + 1)],
                                start=False, stop=False,
                            )
                    i_stop = None
                    for chunks in (CH[L], CH[R]):
                        for k in KS[L]:
                            for m in chunks:
                                p_, c0 = _mm_dest(ps[0], ps[1], m)
                                last = k == KS[L][-1] and m == chunks[-1]
                                if PS_ONE:
                                    stop_ = last and chunks is CH[R]
                                else:
                                    stop_ = last
                                i_mm = nc.tensor.matmul(
                                    p_[:, c0:c0 + BL],
                                    whh_sb[k][:, 128 * m:128 * (m + 1)],
                                    hT[:, BL * k:BL * (k + 1)],
                                    start=False, stop=stop_,
                                )
                                if stop_:
                                    i_stop = i_mm

                    hT_new = hTp.tile([128, 4 * BL], BF16, tag="hT", name="hT")
                    if C_FRESH:
                        c_new = hTp.tile([128, 4 * BL], F32, tag="cT",
                                         name="cT")
                    else:
                        c_new = c_cur
                    if TAIL_SCAN and PS_ONE:
                        # one sig tile for both halves: half hf at cols
                        # [256*hf, 256*hf+256), gates at even cols (zeros
                        # preset at odd cols).
                        sg = sigp.tile([128, 512], F32, tag="sig", name="sig")
                        sgv = sg[:].rearrange("p (h j t) -> p h j t",
                                              h=2, t=2)
                        i_sL = nc.scalar.activation(
                            sgv[:, L, 0:96, 0], ps[L][:, 128 * L:128 * L + 96],
                            AF.Sigmoid)
                        i_sR = nc.scalar.activation(
                            sgv[:, R, 0:96, 0], ps[R][:, 128 * R:128 * R + 96],
                            AF.Sigmoid)
                        psv = ps[0][:].rearrange("p (h c) -> p h c", h=2)
                        i_so = nc.scalar.activation(
                            sgv[:, :, 96:128, 0], psv[:, :, 96:128],
                            AF.Sigmoid)
                        # sub-range gating only (same PE-write/ACT-read bank
                        # overlap the 2-bank baseline already runs with on
                        # HW).  SIG_STOPDEP restores strict full-group
                        # gating if needed.
                        if SIG_STOPDEP and i_stop is not None:
                            i_sL.ins.add_dependency(
                                i_stop.ins.name,
                                mybir.DependencyInfo(sync=True, no_sync=False))
                            i_sR.ins.add_dependency(
                                i_sL.ins.name,
                                mybir.DependencyInfo(sync=False, no_sync=True))
                            i_so.ins.add_dependency(
                                i_sR.ins.name,
                                mybir.DependencyInfo(sync=False, no_sync=True))
                        sigs = {L: None, R: None}
                        for hf in (L, R):
                            sv = sgv[:, hf]
                            ci_ = cil[hf]
                            cv = ci_[:].rearrange("p (j t) -> p j t", t=2)
                            i_u = nc.vector.scalar_tensor_tensor(
                                cv[:, 1:33, 0], sv[:, 64:96, 0], -0.5,
                                sv[:, 0:32, 0], OP.add, OP.mult,
                            )
                            i_c = nc.vector.tensor_tensor_scan(
                                ci_[:, 0:64], sg[:, 256 * hf + 63:256 * hf + 127],
                                ci_[:, 1:65], 0.0, OP.mult, OP.add,
                            )
                            _demote(i_c, i_u)
                            sc = smlp.tile([128, 32], F32, tag=f"sc{hf}",
                                           name=f"sc{hf}")
                            nc.scalar.activation(sc[:], cv[:, 0:32, 1],
                                                 AF.Sigmoid, scale=4.0)
                            if HH_SPLIT:
                                for q in range(2):
                                    Sq = slice(32 * hf + 16 * q,
                                               32 * hf + 16 * (q + 1))
                                    nc.vector.scalar_tensor_tensor(
                                        hT_new[:, Sq],
                                        sc[:, 16 * q:16 * (q + 1)], -0.5,
                                        sv[:, 96 + 16 * q:96 + 16 * (q + 1), 0],
                                        OP.add, OP.mult,
                                    )
                            else:
                                nc.vector.scalar_tensor_tensor(
                                    hT_new[:, 32 * hf:32 * (hf + 1)], sc[:],
                                    -0.5, sv[:, 96:128, 0], OP.add, OP.mult,
                                )
                        hT = hT_new
                        continue
                    if TAIL_SCAN:
                        if SCO_FUSE:
                            # o-gates (/4) -> odd cols 67..129 of the WRITE
                            # buffer.  Emitted BEFORE the sigmoids so the
                            # DVE queue reaches them before any
                            # sigma-dependent wait; with o-gate chunks
                            # ordered first their psum cols are ready
                            # during the whh block.
                            for hf in (L, R):
                                cvh = cil[hf][(t + 1) % 2][:].rearrange(
                                    "p (j t2) -> p j t2", t2=2)
                                nc.vector.tensor_scalar_mul(
                                    cvh[:, 33:65, 1], ps[hf][:, 96:128], 0.25)
                        sigs = {}
                        # phase 1: ifg sigmoids (ACT in-order; lead first).
                        # sig layout: gates at even cols (zeros preset at
                        # odd cols).
                        for hf in (L, R):
                            s_ = sigp.tile([128, 256], F32, tag=f"sig{hf}",
                                           name=f"sig{hf}")
                            sv = s_[:].rearrange("p (j t) -> p j t", t=2)
                            nc.scalar.activation(sv[:, 0:96, 0],
                                                 ps[hf][:, 0:96],
                                                 AF.Sigmoid)
                            if not SCO_FUSE:
                                nc.scalar.activation(sv[:, 96:128, 0],
                                                     ps[hf][:, 96:128],
                                                     AF.Sigmoid)
                            sigs[hf] = s_
                        # phase 2: per-half u + c-scan + sigma(4x) + h
                        for hf in (L, R):
                            s_ = sigs[hf]
                            sv = s_[:].rearrange("p (j t2) -> p j t2", t2=2)
                            ca = cil[hf][t % 2]
                            cb = cil[hf][(t + 1) % 2]
                            cv_a = ca[:].rearrange("p (j t2) -> p j t2", t2=2)
                            cv = cb[:].rearrange("p (j t2) -> p j t2", t2=2)
                            # u_j -> even cols 2+2j of the read buffer
                            i_u = nc.vector.scalar_tensor_tensor(
                                cv_a[:, 1:33, 0], sv[:, 64:96, 0], -0.5,
                                sv[:, 0:32, 0], OP.add, OP.mult,
                            )
                            # c~_j = sigf_j * c~_j + u_j  (one scan op;
                            # d0 = sig cols 63..126 = [0, f_0, 0, f_1, ...])
                            i_c = nc.vector.tensor_tensor_scan(
                                cb[:, 0:64], s_[:, 63:127], ca[:, 1:65],
                                0.0, OP.mult, OP.add,
                            )
                            _demote(i_c, i_u)
                            if SCO_FUSE:
                                sco = smlp.tile([128, 65], F32,
                                                tag=f"sc{hf}", name=f"sc{hf}")
                                nc.scalar.activation(sco[:], cv[:, 0:65, 1],
                                                     AF.Sigmoid, scale=4.0)
                                if HH_SPLIT:
                                    for q in range(2):
                                        Sq = slice(32 * hf + 16 * q,
                                                   32 * hf + 16 * (q + 1))
                                        nc.vector.scalar_tensor_tensor(
                                            hT_new[:, Sq],
                                            sco[:, 16 * q:16 * (q + 1)], -0.5,
                                            sco[:, 33 + 16 * q:49 + 16 * q],
                                            OP.add, OP.mult,
                                        )
                                else:
                                    nc.vector.scalar_tensor_tensor(
                                        hT_new[:, 32 * hf:32 * (hf + 1)],
                                        sco[:, 0:32], -0.5, sco[:, 33:65],
                                        OP.add, OP.mult,
                                    )
                                continue
                            sc = smlp.tile([128, 32], F32, tag=f"sc{hf}",
                                           name=f"sc{hf}")
                            nc.scalar.activation(sc[:], cv[:, 0:32, 1],
                                                 AF.Sigmoid, scale=4.0)
                            if HH_SPLIT:
                                for q in range(2):
                                    Sq = slice(32 * hf + 16 * q,
                                               32 * hf + 16 * (q + 1))
                                    nc.vector.scalar_tensor_tensor(
                                        hT_new[:, Sq],
                                        sc[:, 16 * q:16 * (q + 1)], -0.5,
                                        sv[:, 96 + 16 * q:96 + 16 * (q + 1), 0],
                                        OP.add, OP.mult,
                                    )
                            else:
                                nc.vector.scalar_tensor_tensor(
                                    hT_new[:, 32 * hf:32 * (hf + 1)], sc[:],
                                    -0.5, sv[:, 96:128, 0], OP.add, OP.mult,
                                )
                        hT = hT_new
                        if C_FRESH:
                            c_cur = c_new
                        continue
                    sigs = {}
                    # sigmoids first (ACT is in-order; lead half first)
                    for hf in (L, R):
                        s_ = sigp.tile([128, 128], F32, tag=f"sig{hf}",
                                       name=f"sig{hf}")
                        # i,f,g first (gates the c chain), o later
                        nc.scalar.activation(s_[:, 0:96], ps[hf][:, 0:96],
                                             AF.Sigmoid)
                        nc.scalar.activation(s_[:, 96:128],
                                             ps[hf][:, 96:128], AF.Sigmoid)
                        sigs[hf] = s_
                        S = slice(32 * hf, 32 * (hf + 1))
                        if TAIL_DVE:
                            # v, u, c engine-chained on DVE; c's RAW deps on
                            # u and v are enforced by DVE program order, so
                            # the sems are demoted to nosync edges.
                            v = smlp.tile([128, 32], F32, tag=f"v{hf}",
                                          name=f"v{hf}")
                            i_v = nc.vector.tensor_mul(
                                v[:], s_[:, 32:64], c_cur[:, S])
                            u = smlp.tile([128, 32], F32, tag=f"u{hf}",
                                          name=f"u{hf}")
                            i_u = nc.vector.scalar_tensor_tensor(
                                u[:], s_[:, 64:96], -0.5, s_[:, 0:32],
                                OP.add, OP.mult,
                            )
                            i_c = nc.vector.scalar_tensor_tensor(
                                c_new[:, S], u[:], 2.0, v[:],
                                OP.mult, OP.add,
                            )
                            _demote(i_c, i_u, i_v)
                        else:
                            # c update for this half (only TensorTensor is
                            # legal on Pool, so at most the v-multiply can
                            # be offloaded there)
                            v_pool = (POOL_V == 2
                                      or (POOL_V == 1 and hf == R)
                                      or (POOL_V == 3 and hf == L))
                            u = smlp.tile([128, 32], F32, tag=f"u{hf}",
                                          name=f"u{hf}")
                            nc.vector.scalar_tensor_tensor(
                                u[:], s_[:, 64:96], -0.5, s_[:, 0:32],
                                OP.add, OP.mult,
                            )
                            v = smlp.tile([128, 32], F32, tag=f"v{hf}",
                                          name=f"v{hf}")
                            veng = nc.gpsimd if v_pool else nc.vector
                            veng.tensor_mul(v[:], s_[:, 32:64], c_cur[:, S])
                            nc.vector.scalar_tensor_tensor(
                                c_new[:, S], u[:], 2.0, v[:],
                                OP.mult, OP.add,
                            )
                    for hf in (L, R):
                        S = slice(32 * hf, 32 * (hf + 1))
                        sc = smlp.tile([128, 32], F32, tag=f"sc{hf}",
                                       name=f"sc{hf}")
                        nc.scalar.activation(sc[:], c_new[:, S], AF.Sigmoid,
                                             scale=2.0)
                        if HH_SPLIT:
                            for q in range(2):
                                Sq = slice(32 * hf + 16 * q,
                                           32 * hf + 16 * (q + 1))
                                nc.vector.scalar_tensor_tensor(
                                    hT_new[:, Sq], sc[:, 16 * q:16 * (q + 1)],
                                    -0.5, sigs[hf][:, 96 + 16 * q:96 + 16 * (q + 1)],
                                    OP.add, OP.mult,
                                )
                        else:
                            nc.vector.scalar_tensor_tensor(
                                hT_new[:, S], sc[:], -0.5, sigs[hf][:, 96:128],
                                OP.add, OP.mult,
                            )
                    hT = hT_new
                    if C_FRESH:
                        c_cur = c_new
                while early_emitted < len(early_q):
                    early_q[early_emitted]()
                    early_emitted += 1
                for seg in conv_segs:
                    th = seg[0]
                    while seg[3] < len(th):
                        th[seg[3]]()
                        seg[3] += 1

            # ================= Head ========================================
            with (
                tc.tile_pool(name="hd", bufs=1) as hd,
                tc.tile_pool(name="hps", bufs=1, space="PSUM") as hpsp,
            ):
                hps = hpsp.tile([64, BL], F32, tag="hps")
                nc.tensor.matmul(hps[:], b1_sb[:], ones_sb[:],
                                 start=True, stop=False)
                for k in range(4):
                    nc.tensor.matmul(
                        hps[:], w1T_sb[k][:], hT[:, BL * k:BL * (k + 1)],
                        start=False, stop=(k == 3),
                    )
                hid = hd.tile([64, BL], BF16, tag="hid")
                nc.scalar.activation(hid[:], hps[:], AF.Relu)
                lps = hpsp.tile([2, BL], F32, tag="lps")
                nc.tensor.matmul(lps[:], b2_sb[:], ones_sb[:],
                                 start=True, stop=False)
                nc.tensor.matmul(lps[:], w2T_sb[:], hid[:],
                                 start=False, stop=True)
                outt = hd.tile([2, BL], F32, tag="outt")
                nc.vector.tensor_copy(outt[:], lps[:])
                nc.sync.dma_start(out[:], outt[:])

    nc.compile()
    return nc


def _prep(inputs):
    x = np.asarray(inputs["x"], np.float32)
    conv_w = np.asarray(inputs["conv_w"], np.float32)
    bn_gamma = np.asarray(inputs["bn_gamma"], np.float32)
    bn_beta = np.asarray(inputs["bn_beta"], np.float32)
    w_ih = np.asarray(inputs["w_ih"], np.float32)
    w_hh = np.asarray(inputs["w_hh"], np.float32)
    b_ih = np.asarray(inputs["b_ih"], np.float32)
    b_hh = np.asarray(inputs["b_hh"], np.float32)
    w1 = np.asarray(inputs["w1"], np.float32)
    b1 = np.asarray(inputs["b1"], np.float32)
    w2 = np.asarray(inputs["w2"], np.float32)
    b2 = np.asarray(inputs["b2"], np.float32)
    bf = ml_dtypes.bfloat16

    # ---- BN batch statistics (host, exact) ----
    xp_ = np.pad(x, ((0, 0), (0, 0), (2, 2)))
    Xt = np.ascontiguousarray(xp_.transpose(1, 0, 2))  # [C, B, T+4]
    acc = np.zeros((C, B, T), np.float32)
    for k in range(5):
        acc += np.tensordot(conv_w[:, :, k], Xt[:, :, k:k + T], axes=(1, 0))
    mean = acc.mean(axis=(1, 2), dtype=np.float64)
    var = (acc.astype(np.float64) ** 2).mean(axis=(1, 2)) - mean ** 2
    bn_a = (bn_gamma.astype(np.float64) / np.sqrt(var + EPS))
    bn_b = bn_beta.astype(np.float64) - mean * bn_a
    bn_ab = np.stack([bn_a, bn_b], axis=1).astype(np.float32)  # [C, 2]

    # ---- gate permutation: [i | f | o | g] with g rows scaled x2 ----
    perm = np.r_[0:512, 512:1024, 1536:2048, 1024:1536]
    rs = np.ones((G, 1), np.float32)
    rs[1536:2048] = 2.0

    w_ih_p = w_ih[perm] * rs                       # [G, C]
    w_hh_p = w_hh[perm] * rs * 2.0                 # [G, H]
    bias_p = ((b_ih + b_hh)[perm] * rs[:, 0])      # [G]

    wihT = np.ascontiguousarray(w_ih_p.T.reshape(2, 128, G)).astype(bf)
    whhT = np.ascontiguousarray(w_hh_p.T.reshape(4, 128, G)).astype(bf)

    bias4 = bias_p.reshape(4, 4, 128)[[0, 1, 3, 2]]  # col order [i,f,g,o]
    biasT = np.stack([
        bias4[:, 0:2, :].reshape(8, 128),
        bias4[:, 2:4, :].reshape(8, 128),
    ]).astype(bf)                                   # [half, j=col*2+s2, gp]
    maskb = np.zeros((8, 128), np.float32)
    for j in range(8):
        maskb[j, 16 * j:16 * (j + 1)] = 1.0
    maskb = maskb.astype(bf)
    # single-tile variants: j = 8*half + GT2COL[gt]*2 + s2 -> 16-col block
    biasTf = np.zeros((16, 128), np.float32)
    for j in range(16):
        h_, r = j // 8, j % 8
        gt = GT2COL.index(r // 2)
        m = gt * 4 + 2 * h_ + (r % 2)
        biasTf[j] = bias_p[128 * m:128 * (m + 1)]
    biasTf = biasTf.astype(bf)
    maskf = np.zeros((16, 256), np.float32)
    for j in range(16):
        maskf[j, 16 * j:16 * (j + 1)] = 1.0
    maskf = maskf.astype(bf)

    cdt = bf if CONV_BF16 else np.float32
    cw = np.zeros((10, 128, 256), np.float32)
    for k in range(5):
        for cc in range(2):
            cw[k * 2 + cc] = conv_w[:, 128 * cc:128 * (cc + 1), k].T
    if EVICT_DVE:
        # fold the BN scale into the conv weights (per output channel =
        # last axis of cw); eviction then only needs (+bias, relu)
        cw = cw * bn_a.astype(np.float32)[None, None, :]
    cw = cw.astype(cdt)

    w1T = np.ascontiguousarray((2.0 * w1).T.reshape(4, 128, 64)).astype(bf)
    w2T = np.ascontiguousarray(w2.T).astype(bf)

    common = dict(
        cw=cw,
        bn_ab=bn_ab,
        wihT=wihT,
        whhT=whhT,
        biasT=biasT,
        maskb=maskb,
        biasTf=biasTf,
        maskf=maskf,
        w1T=w1T,
        b1r=b1.reshape(1, 64).astype(bf),
        w2T=w2T,
        b2r=b2.reshape(1, 2).astype(bf),
        ones1=np.ones((1, 16), bf),
    )
    in_maps = []
    x_c = x.astype(cdt)
    for core in range(NCORES):
        m = dict(common)
        m["x_l"] = np.ascontiguousarray(x_c[BL * core:BL * (core + 1)])
        in_maps.append(m)
    return in_maps


def kernel(**inputs) -> np.ndarray:
    if "nc" not in _cache:
        _cache["nc"] = _build()
    nc = _cache["nc"]
    in_maps = _prep(inputs)
    res = run_bass_kernel_spmd(nc, in_maps, list(range(NCORES)))
    _cache["last_res"] = res
    return np.concatenate([res.results[c]["out"].T for c in range(NCORES)],
                          axis=0).astype(np.float32)



# revision 46
# speedup vs baseline: 1.0013x; 1.0001x over previous
"""CNN-LSTM decoder Trainium2 kernel (8 NeuronCores, data-parallel over batch).

Model (see reference): Conv1d(256->256,k=5,p=2) -> BatchNorm1d(train stats) ->
ReLU -> LSTM(256->512, T=1024) -> Linear(512->64) -> ReLU -> Linear(64->2).

Distribution: batch 128 split 16/core across 8 cores.  Per core:

  Conv:    time-block tiled ([128, w+4] x slices, N=w bf16 matmuls,
           BN+ReLU fused into the ACT eviction straight into a persistent
           SBUF buffer featsT[cc][ci, t, b] in bf16).  Only the first 32
           output cols are computed before the scan starts; the rest is
           emitted into the scan loop against per-piece consumption
           deadlines and hidden in engine idle gaps by the Tile priority
           scheduler.  BN batch stats are computed host-side.

  Scan:    fully transposed layout.  Gates live as gatesT [128 gate-dims,
           16 batch] PSUM tiles, two halves (H-slices 0-1 / 2-3, one PSUM
           bank each); psum cols = [i|f|g|o]*32 + s2*16 + b.  Per step:
             - 2 bias matmuls (K=8 mask trick) open each half's accum group
             - 32 input-projection matmuls (wihT chunks @ featsT[:, t, :]),
               emitted one step ahead so they fill PE idle time
             - 64 recurrent matmuls (whhT chunks @ hT k-slices), N=16 bf16
           Tail per half (TAIL_SCAN): one sigmoid over i/f/g writing to
           EVEN columns of the sig tile (odd columns preset to zero; g rows
           pre-scaled x2 so tanh(x) = 2*sigmoid(2x)-1), sigmoid over o,
           then u = (sig2g-.5)*sigi into the even cols of an interleaved
           [., c~, u, c~, u, ...] buffer and the whole c-update
           c~_j = sigf_j*c~_j + u_j as ONE DVE tensor_tensor_scan
           (2-element cells [reset, apply]; d0 = [0|sigf] interleaved via
           the preset zeros, ping-pong buffers per step parity).  c~ = c/2,
           so sigma(2c) = sigma(4c~).  h = (sig(4c~)-.5)*sigo is written
           as h/2 in bf16, the exact moving operand of the next step's
           recurrent matmuls.  Same-engine RAW deps in the tail are demoted
           to nosync edges (_demote): in-order engines make the semaphores
           redundant, removing their latency from the recurrence chain.
           The half whose tail finished late leads the next step
           (alternating), keeping both ACT chains near-saturated.  The
           cycle is bound by the dense ACT sequence (2 sigma96 + 2 sigma_o
           + 2 sigma(4c~)) plus the psum-stop coupling.

  Head:    transposed too: hid [64, 16] = relu(w1 @ h + b1); out [2, 16].

All host-side weight massaging (permutation, scaling, transposes, BN stats)
happens in kernel() below; the device kernel is compiled once per process.
"""

import sys

sys.path.insert(0, "/opt/trn_rl_repo")

import ml_dtypes
import numpy as np

import concourse.bass as bass
import concourse.tile as tile
from concourse import bacc, mybir
from concourse.bass_utils import run_bass_kernel_spmd

F32 = mybir.dt.float32
F32R = mybir.dt.float32r
BF16 = mybir.dt.bfloat16
AF = mybir.ActivationFunctionType
OP = mybir.AluOpType

B, C, T, H = 128, 256, 1024, 512
G = 4 * H  # 2048
NCORES = 8
BL = B // NCORES  # 16 examples per core
EPS = 1e-5

# gate chunk m = gt*4 + s (gt in [i,f,o,g], s = H-slice 0..3)
# psum half = s // 2; col within half = gt*32 + (s % 2)*16 + b
# within each half, o-gate chunks (gt==2 in perm order) go last so the
# ifg sigmoid is not gated on the o-column matmuls
A_CHUNKS = ([m for m in range(16) if (m % 4) < 2 and m // 4 != 2]
            + [m for m in range(16) if (m % 4) < 2 and m // 4 == 2])
B_CHUNKS = ([m for m in range(16) if (m % 4) >= 2 and m // 4 != 2]
            + [m for m in range(16) if (m % 4) >= 2 and m // 4 == 2])

PSUM_PAD = True
ALT_LEAD = True
HH_SPLIT = True
C_FRESH = False
POOL_V = 1
KORD = 0
SGROUP = 0
EVICT_DVE = False
CPS_BUFS = 3
DL_MARGIN = 0
HT_BUFS = 2
SIG_BUFS = 2
PS_BUFS = 2
PRE0 = 32
SIG_SPLIT = 2
CONV_STREAM = True
CONV_BF16 = True
DEMOTE = True   # demote same-engine data deps to nosync (engine order
                # guarantees RAW on HW: in-order engines drain the pipe
                # between instructions)
TAIL_DVE = True   # (pre-scan fallback) v on DVE ahead of u/c, engine-chained
TAIL_SCAN = True  # c-update as one tensor_tensor_scan over the interleaved
                  # [c~|u] ping-pong buffers (c~ = c/2)
SIG_MERGE = False   # (tried) merge sigma_ifg+sigma_o: breaks phase stagger
PS_ONE = False      # (tried) both halves in one psum bank: re-phases badly
SIG_STOPDEP = False
CONV_SEGS = ((128, 192), (320, 192), (512, 192), (704, 192), (896, 128))
EARLY_DL = 90
EARLY_T0 = 10   # don't emit early conv pieces before this step
XP_BUFS = 6
SCO_FUSE = False    # (tried) fuse sigma_o into sigma(4c~): o-copy lands on
                    # the DVE critical chain

_cache = {}


def _demote(cons, *prods):
    """Demote cons's sync deps on prods to nosync deps.

    Only valid when cons and every prod run on the same engine: the
    scheduler still enforces ordering, and in-order engine execution
    makes the RAW safe without a semaphore."""
    if not DEMOTE:
        return
    ci = cons.ins
    sync = ci.sync_dependency_set_copy()
    for p in prods:
        pi = p.ins
        assert pi.engine == ci.engine, (pi.engine, ci.engine)
        if pi.name in sync:
            ci.remove_dependency(pi.name)
            ci.add_dependency(
                pi.name, mybir.DependencyInfo(sync=False, no_sync=True)
            )


GT2COL = (0, 1, 3, 2)  # perm order [i,f,o,g] -> col blocks [i,f,g,o]


def _mm_dest(psA, psB, m):
    gt, s = m // 4, m % 4
    if PS_ONE:
        c0 = 128 * (s // 2) + GT2COL[gt] * 32 + (s % 2) * 16
        return psA, c0
    ps = psA if s < 2 else psB
    c0 = GT2COL[gt] * 32 + (s % 2) * 16
    return ps, c0


def _build(nT=T, skip_conv=False):
    CDT = BF16 if CONV_BF16 else F32R
    nc = bacc.Bacc("TRN2", target_bir_lowering=False, debug=False,
                   num_devices=NCORES)

    x_l = nc.dram_tensor("x_l", [BL, C, T], CDT, kind="ExternalInput").ap()
    cw = nc.dram_tensor("cw", [10, 128, 256], CDT, kind="ExternalInput").ap()
    bn_ab = nc.dram_tensor("bn_ab", [C, 2], F32, kind="ExternalInput").ap()
    wihT = nc.dram_tensor("wihT", [2, 128, G], BF16, kind="ExternalInput").ap()
    whhT = nc.dram_tensor("whhT", [4, 128, G], BF16, kind="ExternalInput").ap()
    biasT = nc.dram_tensor("biasT", [2, 8, 128], BF16, kind="ExternalInput").ap()
    maskb = nc.dram_tensor("maskb", [8, 128], BF16, kind="ExternalInput").ap()
    biasTf = nc.dram_tensor("biasTf", [16, 128], BF16, kind="ExternalInput").ap()
    maskf = nc.dram_tensor("maskf", [16, 256], BF16, kind="ExternalInput").ap()
    w1T = nc.dram_tensor("w1T", [4, 128, 64], BF16, kind="ExternalInput").ap()
    b1r = nc.dram_tensor("b1r", [1, 64], BF16, kind="ExternalInput").ap()
    w2T = nc.dram_tensor("w2T", [64, 2], BF16, kind="ExternalInput").ap()
    b2r = nc.dram_tensor("b2r", [1, 2], BF16, kind="ExternalInput").ap()
    ones1 = nc.dram_tensor("ones1", [1, 16], BF16, kind="ExternalInput").ap()

    out = nc.dram_tensor("out", [2, BL], F32, kind="ExternalOutput").ap()

    if SCO_FUSE:
        # o-gate chunks first: their psum cols finish early so the DVE
        # o-copies run during the whh block, fully hidden
        A_CH = ([m for m in range(16) if (m % 4) < 2 and m // 4 == 2]
                + [m for m in range(16) if (m % 4) < 2 and m // 4 != 2])
        B_CH = ([m for m in range(16) if (m % 4) >= 2 and m // 4 == 2]
                + [m for m in range(16) if (m % 4) >= 2 and m // 4 != 2])
    else:
        A_CH, B_CH = A_CHUNKS, B_CHUNKS

    with tile.TileContext(nc) as tc:
        with (
            tc.tile_pool(name="const", bufs=1) as const,
            tc.tile_pool(name="state", bufs=1) as state,
        ):
            # ---- persistent constants in SBUF ----
            cwb = const.tile([128, 10 * 256], CDT, tag="cwb")
            cw_sb = [cwb[:, 256 * i:256 * (i + 1)] for i in range(10)]
            nc.sync.dma_start(
                cwb[:].rearrange("p (n c) -> p n c", n=10),
                cw[0:10].transpose([1, 0, 2]),
            )
            bn_sb = [const.tile([128, 2], F32, name=f"bn{i}", tag=f"bn{i}")
                     for i in range(2)]
            for i in range(2):
                nc.sync.dma_start(bn_sb[i][:], bn_ab[128 * i:128 * (i + 1), :])
            # (DMAs for the tensors below are emitted after the conv
            # prelude so the prelude's x slices go first in the DMA queue;
            # they only have to arrive before scan step 0's matmuls.)
            wihb = const.tile([128, 2 * G], BF16, tag="wihb")
            wih_sb = [wihb[:, G * i:G * (i + 1)] for i in range(2)]
            whhb = const.tile([128, 4 * G], BF16, tag="whhb")
            whh_sb = [whhb[:, G * i:G * (i + 1)] for i in range(4)]
            biasT_sb = [const.tile([8, 128], BF16, name=f"biasT{i}",
                                   tag=f"biasT{i}") for i in range(2)]
            mask_sb = const.tile([8, 128], BF16, tag="mask_sb")
            biasTf_sb = const.tile([16, 128], BF16, tag="biasTf_sb")
            maskf_sb = const.tile([16, 256], BF16, tag="maskf_sb")
            w1T_sb = [const.tile([128, 64], BF16, name=f"w1T{i}", tag=f"w1T{i}")
                      for i in range(4)]
            b1_sb = const.tile([1, 64], BF16, tag="b1_sb")
            w2T_sb = const.tile([64, 2], BF16, tag="w2T_sb")
            b2_sb = const.tile([1, 2], BF16, tag="b2_sb")
            ones_sb = const.tile([1, 16], BF16, tag="ones_sb")

            def load_scan_weights():
                nc.sync.dma_start(
                    wihb[:].rearrange("p (n g) -> p n g", n=2),
                    wihT[0:2].transpose([1, 0, 2]),
                )
                nc.sync.dma_start(
                    whhb[:].rearrange("p (n g) -> p n g", n=4),
                    whhT[0:4].transpose([1, 0, 2]),
                )
                if PS_ONE:
                    nc.sync.dma_start(biasTf_sb[:], biasTf[:])
                    nc.sync.dma_start(maskf_sb[:], maskf[:])
                else:
                    for i in range(2):
                        nc.sync.dma_start(biasT_sb[i][:], biasT[i])
                    nc.sync.dma_start(mask_sb[:], maskb[:])
                for i in range(4):
                    nc.sync.dma_start(w1T_sb[i][:], w1T[i])
                nc.sync.dma_start(b1_sb[:], b1r[:])
                nc.sync.dma_start(w2T_sb[:], w2T[:])
                nc.sync.dma_start(b2_sb[:], b2r[:])
                nc.sync.dma_start(ones_sb[:], ones1[:])

            # ---- persistent activations / state ----
            featsT = [state.tile([128, T, BL], BF16, name=f"featsT{i}",
                                 tag=f"featsT{i}") for i in range(2)]
            c_st = state.tile([128, 4 * BL], F32, tag="c_st")
            nc.vector.memset(c_st[:], 0.0)
            # TAIL_SCAN: per-half interleaved [., c~_0, u_0, c~_1, u_1, ...]
            # buffer (c~ = c/2 at odd cols 1+2j, u at even cols 2+2j).  The
            # c-recurrence c~_j = sigmf_j * c~_j + u_j is ONE DVE
            # tensor_tensor_scan per half: 2-element cells [reset, apply]
            # with d0 = [0, sigmf] (zeros preset at odd cols of the sig
            # tile), d1 = this buffer.
            # ping-pong per step parity: scan(t) reads buf[t%2], writes
            # buf[(t+1)%2]; the o-copy also targets the write buffer
            cil = [[state.tile([128, 132], F32, name=f"cil{h_}{p_}",
                               tag=f"cil{h_}{p_}") for p_ in range(2)]
                   for h_ in range(2)]
            for h_ in range(2):
                for p_ in range(2):
                    nc.vector.memset(cil[h_][p_][:], 0.0)
            # C_FRESH: c lives in a rotating pool instead (see scan loop)
            if skip_conv:
                for i in range(2):
                    nc.vector.memset(featsT[i][:].bitcast(F32), 0.0)

            # ===== Conv (tq-streamed into the scan) + Scan =================
            # conv tiled as 8 time-blocks of 128; block 0 is a short
            # prelude, blocks 1..7 are emitted into the scan loop at a
            # uniform rate so block j is ready before scan step 128*j.
            # The Tile priority scheduler slots the ops into idle engine
            # gaps (PE ~60%, ACT ~50% idle during the scan).
            def conv_piece_thunks(t0, w):
                # conv for output cols [t0, t0+w) over all examples/channels
                thunks = []
                if skip_conv:
                    return thunks
                xts = {}

                def load_x(ex, t0=t0, w=w):
                    lo = t0 - 2
                    pair = []
                    for cc in range(2):
                        t_ = xp.tile([128, w + 4], CDT, name=f"xs{cc}",
                                     tag=f"xs{cc}")
                        s0, s1 = max(lo, 0), min(lo + w + 4, T)
                        if t0 == 0:
                            nc.vector.memset(t_[:, 0:2].bitcast(F32), 0.0)
                        if t0 + w == T:
                            nc.vector.memset(
                                t_[:, w + 2:w + 4].bitcast(F32), 0.0)
                        nc.sync.dma_start(
                            t_[:, s0 - lo:s1 - lo],
                            x_l[ex, 128 * cc:128 * (cc + 1), s0:s1],
                        )
                        pair.append(t_)
                    xts[ex] = pair

                for ex in range(BL):
                    thunks.append(lambda ex=ex: load_x(ex))
                    for co in range(2):
                        cst = {}

                        def mk_mm(ex, co, cc, k, first, w=w, cst=cst):
                            def mm():
                                if first:
                                    cst["ps"] = cpsp.tile([128, w], F32,
                                                          tag="cps",
                                                          name="cps",
                                                          padded_shape=[128, 512])
                                nc.tensor.matmul(
                                    cst["ps"][:],
                                    cw_sb[k * 2 + cc][:, 128 * co:128 * (co + 1)],
                                    xts[ex][cc][:, k:k + w],
                                    start=first,
                                    stop=(cc == 1 and k == 4),
                                )
                            return mm

                        first = True
                        for cc in range(2):
                            for k in range(5):
                                thunks.append(mk_mm(ex, co, cc, k, first))
                                first = False

                        def evict(ex=ex, co=co, t0=t0, w=w, cst=cst):
                            if EVICT_DVE:
                                # bn scale folded into conv weights host-side
                                nc.vector.tensor_scalar(
                                    featsT[co][:, t0:t0 + w, ex],
                                    cst["ps"][:], bn_sb[co][:, 1:2], 0.0,
                                    OP.add, OP.max,
                                )
                            else:
                                nc.scalar.activation(
                                    featsT[co][:, t0:t0 + w, ex],
                                    cst["ps"][:], AF.Relu,
                                    bias=bn_sb[co][:, 1:2],
                                    scale=bn_sb[co][:, 0:1],
                                )
                        thunks.append(evict)
                return thunks

            # ================= Scan (transposed layout) ====================
            with (
                tc.tile_pool(name="xp", bufs=XP_BUFS) as xp,
                tc.tile_pool(name="cps", bufs=CPS_BUFS, space="PSUM") as cpsp,
                tc.tile_pool(name="hTp", bufs=HT_BUFS) as hTp,
                tc.tile_pool(name="sig", bufs=SIG_BUFS) as sigp,
                tc.tile_pool(name="sml", bufs=SIG_BUFS) as smlp,
                tc.tile_pool(name="psA", bufs=PS_BUFS, space="PSUM") as psAp,
                tc.tile_pool(name="psB", bufs=PS_BUFS, space="PSUM") as psBp,
            ):
                # conv prelude: only the first PRE0 cols must precede
                # step 0; the rest of block 0 streams into the first steps
                # with tight deadlines, blocks 1..7 at a uniform rate.
                # prelude piece with the x loads batched into one wide
                # DMA per cc (32 separate dma_starts would serialize ~18us
                # of SP sequencer time before the first conv matmul)
                if not skip_conv:
                    wp = PRE0 + 4
                    xb = [xp.tile([128, BL * wp], CDT, name=f"xb{cc}",
                                  tag=f"xb{cc}") for cc in range(2)]
                    for cc in range(2):
                        xv = xb[cc][:].rearrange("p (e w) -> p e w", e=BL)
                        nc.vector.memset(xv[:, :, 0:2].bitcast(F32), 0.0)
                        nc.sync.dma_start(
                            xv[:, :, 2:wp],
                            x_l[0:BL, 128 * cc:128 * (cc + 1),
                                0:PRE0 + 2].transpose([1, 0, 2]),
                        )
                    for ex in range(BL):
                        for co in range(2):
                            pps = cpsp.tile([128, PRE0], F32, tag="cps",
                                            name="cps",
                                            padded_shape=[128, 128])
                            first = True
                            for cc in range(2):
                                for k in range(5):
                                    nc.tensor.matmul(
                                        pps[:],
                                        cw_sb[k * 2 + cc][:, 128 * co:128 * (co + 1)],
                                        xb[cc][:, ex * wp + k:ex * wp + k + PRE0],
                                        start=first,
                                        stop=(cc == 1 and k == 4),
                                    )
                                    first = False
                            if EVICT_DVE:
                                nc.vector.tensor_scalar(
                                    featsT[co][:, 0:PRE0, ex],
                                    pps[:], bn_sb[co][:, 1:2], 0.0,
                                    OP.add, OP.max,
                                )
                            else:
                                nc.scalar.activation(
                                    featsT[co][:, 0:PRE0, ex],
                                    pps[:], AF.Relu,
                                    bias=bn_sb[co][:, 1:2],
                                    scale=bn_sb[co][:, 0:1],
                                )
                load_scan_weights()
                early_q = []
                for t0, w0 in ((PRE0, PRE0), (2 * PRE0, 2 * PRE0)):
                    early_q += conv_piece_thunks(t0, w0)
                early_dl = EARLY_DL
                early_emitted = 0
                # wider mid pieces halve the ACT eviction count; each
                # piece must fully evict before the scan consumes its first
                # column, so each gets its own deadline window.
                conv_segs = []
                prev = 0
                for t0, w in CONV_SEGS:
                    s1 = max(1, min(nT, t0 - 6))
                    conv_segs.append(
                        [conv_piece_thunks(t0, w), prev, s1, 0])
                    prev = s1

                hT = hTp.tile([128, 4 * BL], BF16, tag="hT", name="hT")
                nc.vector.memset(hT[:].bitcast(F32), 0.0)
                if C_FRESH:
                    c_cur = hTp.tile([128, 4 * BL], F32, tag="cT", name="cT")
                    nc.vector.memset(c_cur[:], 0.0)
                else:
                    c_cur = c_st
                if TAIL_SCAN and PS_ONE:
                    # preset the sig-pool buffers' odd columns to zero once;
                    # in-loop writers only touch even columns.
                    for _b in range(SIG_BUFS):
                        s0 = sigp.tile([128, 512], F32, tag="sig", name="sig")
                        sv0 = s0[:].rearrange("p (j t) -> p j t", t=2)
                        nc.vector.memset(sv0[:, :, 1], 0.0)
                elif TAIL_SCAN:
                    for _b in range(SIG_BUFS):
                        for hf in range(2):
                            s0 = sigp.tile([128, 256], F32, tag=f"sig{hf}",
                                           name=f"sig{hf}")
                            sv0 = s0[:].rearrange("p (j t) -> p j t", t=2)
                            nc.vector.memset(sv0[:, :, 1], 0.0)

                CH = (A_CH, B_CH)             # chunks per half
                KS = ((0, 1), (2, 3))         # hT k-slices produced per half
                def open_step(t):
                    # allocate this step's gate psums, open the accumulation
                    # groups with the bias matmuls, and emit the input
                    # projection.  Called one step ahead so these (dependency-
                    # free) matmuls sit ahead of the waiting whh matmuls in
                    # PE's in-order queue and fill its idle time.
                    L = (t % 2) if ALT_LEAD else 0
                    R = 1 - L
                    pshape = [128, 512] if PSUM_PAD else None
                    if PS_ONE:
                        pt = psAp.tile([128, 256], F32, tag="ps", name="ps",
                                       padded_shape=pshape)
                        ps = [pt, pt]
                        nc.tensor.matmul(pt[:], biasTf_sb[:], maskf_sb[:],
                                         start=True, stop=False)
                    else:
                        ps = [None, None]
                        ps[L] = (psAp if L == 0 else psBp).tile(
                            [128, 128], F32, tag=f"ps{L}", name=f"ps{L}",
                            padded_shape=pshape)
                        ps[R] = (psAp if R == 0 else psBp).tile(
                            [128, 128], F32, tag=f"ps{R}", name=f"ps{R}",
                            padded_shape=pshape)
                        nc.tensor.matmul(ps[L][:], biasT_sb[L][:], mask_sb[:],
                                         start=True, stop=False)
                        nc.tensor.matmul(ps[R][:], biasT_sb[R][:], mask_sb[:],
                                         start=True, stop=False)
                    for m in CH[L] + CH[R]:
                        p_, c0 = _mm_dest(ps[0], ps[1], m)
                        for cc in range(2):
                            nc.tensor.matmul(
                                p_[:, c0:c0 + BL],
                                wih_sb[cc][:, 128 * m:128 * (m + 1)],
                                featsT[cc][:, t, :],
                                start=False, stop=False,
                            )
                    return ps

                ps_next = open_step(0)
                for t in range(nT):
                    # stream conv emission at a uniform rate
                    etarget = min(len(early_q),
                                  int(len(early_q) * max(0, t + 1 - EARLY_T0)
                                      / (early_dl - EARLY_T0)))
                    while early_emitted < etarget:
                        early_q[early_emitted]()
                        early_emitted += 1
                    for seg in conv_segs:
                        th, s0, s1, done = seg
                        if t < s0 or done >= len(th):
                            continue
                        tgt = min(len(th),
                                  int(len(th) * (t + 1 - s0) / (s1 - s0)))
                        while seg[3] < tgt:
                            th[seg[3]]()
                            seg[3] += 1
                    # lead half L: its tail (and thus hT slices) finish early;
                    # alternate so the late half of step t leads step t+1.
                    L = (t % 2) if ALT_LEAD else 0
                    R = 1 - L
                    ps = ps_next
                    if t + 1 < nT:
                        ps_next = open_step(t + 1)
                    # recurrent term.  K[R] slices were produced by last
                    # step's lead tail (early) - emit them first; the final
                    # k group is gated by last step's trailing hh.  Within
                    # it, close the lead half's psum first.
                    for k in KS[R]:
                        for m in CH[L] + CH[R]:
                            p_, c0 = _mm_dest(ps[0], ps[1], m)
                            nc.tensor.matmul(
                                p_[:, c0:c0 + BL],
                                whh_sb[k][:, 128 * m:128 * (m + 1)],
                                hT[:, BL * k:BL * (k + 1)],
                                start=False, stop=False,
                            )
                    i_stop = None
                    for chunks in (CH[L], CH[R]):
                        for k in KS[L]:
                            for m in chunks:
                                p_, c0 = _mm_dest(ps[0], ps[1], m)
                                last = k == KS[L][-1] and m == chunks[-1]
                                if PS_ONE:
                                    stop_ = last and chunks is CH[R]
                                else:
                                    stop_ = last
                                i_mm = nc.tensor.matmul(
                                    p_[:, c0:c0 + BL],
                                    whh_sb[k][:, 128 * m:128 * (m + 1)],
                                    hT[:, BL * k:BL * (k + 1)],
                                    start=False, stop=stop_,
                                )
                                if stop_:
                                    i_stop = i_mm

                    hT_new = hTp.tile([128, 4 * BL], BF16, tag="hT", name="hT")
                    if C_FRESH:
                        c_new = hTp.tile([128, 4 * BL], F32, tag="cT",
                                         name="cT")
                    else:
                        c_new = c_cur
                    if TAIL_SCAN and PS_ONE:
                        # one sig tile for both halves: half hf at cols
                        # [256*hf, 256*hf+256), gates at even cols (zeros
                        # preset at odd cols).
                        sg = sigp.tile([128, 512], F32, tag="sig", name="sig")
                        sgv = sg[:].rearrange("p (h j t) -> p h j t",
                                              h=2, t=2)
                        i_sL = nc.scalar.activation(
                            sgv[:, L, 0:96, 0], ps[L][:, 128 * L:128 * L + 96],
                            AF.Sigmoid)
                        i_sR = nc.scalar.activation(
                            sgv[:, R, 0:96, 0], ps[R][:, 128 * R:128 * R + 96],
                            AF.Sigmoid)
                        psv = ps[0][:].rearrange("p (h c) -> p h c", h=2)
                        i_so = nc.scalar.activation(
                            sgv[:, :, 96:128, 0], psv[:, :, 96:128],
                            AF.Sigmoid)
                        # sub-range gating only (same PE-write/ACT-read bank
                        # overlap the 2-bank baseline already runs with on
                        # HW).  SIG_STOPDEP restores strict full-group
                        # gating if needed.
                        if SIG_STOPDEP and i_stop is not None:
                            i_sL.ins.add_dependency(
                                i_stop.ins.name,
                                mybir.DependencyInfo(sync=True, no_sync=False))
                            i_sR.ins.add_dependency(
                                i_sL.ins.name,
                                mybir.DependencyInfo(sync=False, no_sync=True))
                            i_so.ins.add_dependency(
                                i_sR.ins.name,
                                mybir.DependencyInfo(sync=False, no_sync=True))
                        sigs = {L: None, R: None}
                        for hf in (L, R):
                            sv = sgv[:, hf]
                            ci_ = cil[hf]
                            cv = ci_[:].rearrange("p (j t) -> p j t", t=2)
                            i_u = nc.vector.scalar_tensor_tensor(
                                cv[:, 1:33, 0], sv[:, 64:96, 0], -0.5,
                                sv[:, 0:32, 0], OP.add, OP.mult,
                            )
                            i_c = nc.vector.tensor_tensor_scan(
                                ci_[:, 0:64], sg[:, 256 * hf + 63:256 * hf + 127],
                                ci_[:, 1:65], 0.0, OP.mult, OP.add,
                            )
                            _demote(i_c, i_u)
                            sc = smlp.tile([128, 32], F32, tag=f"sc{hf}",
                                           name=f"sc{hf}")
                            nc.scalar.activation(sc[:], cv[:, 0:32, 1],
                                                 AF.Sigmoid, scale=4.0)
                            if HH_SPLIT:
                                for q in range(2):
                                    Sq = slice(32 * hf + 16 * q,
                                               32 * hf + 16 * (q + 1))
                                    nc.vector.scalar_tensor_tensor(
                                        hT_new[:, Sq],
                                        sc[:, 16 * q:16 * (q + 1)], -0.5,
                                        sv[:, 96 + 16 * q:96 + 16 * (q + 1), 0],
                                        OP.add, OP.mult,
                                    )
                            else:
                                nc.vector.scalar_tensor_tensor(
                                    hT_new[:, 32 * hf:32 * (hf + 1)], sc[:],
                                    -0.5, sv[:, 96:128, 0], OP.add, OP.mult,
                                )
                        hT = hT_new
                        continue
                    if TAIL_SCAN:
                        if SCO_FUSE:
                            # o-gates (/4) -> odd cols 67..129 of the WRITE
                            # buffer.  Emitted BEFORE the sigmoids so the
                            # DVE queue reaches them before any
                            # sigma-dependent wait; with o-gate chunks
                            # ordered first their psum cols are ready
                            # during the whh block.
                            for hf in (L, R):
                                cvh = cil[hf][(t + 1) % 2][:].rearrange(
                                    "p (j t2) -> p j t2", t2=2)
                                nc.vector.tensor_scalar_mul(
                                    cvh[:, 33:65, 1], ps[hf][:, 96:128], 0.25)
                        sigs = {}
                        # phase 1: ifg sigmoids (ACT in-order; lead first).
                        # sig layout: gates at even cols (zeros preset at
                        # odd cols).
                        for hf in (L, R):
                            s_ = sigp.tile([128, 256], F32, tag=f"sig{hf}",
                                           name=f"sig{hf}")
                            sv = s_[:].rearrange("p (j t) -> p j t", t=2)
                            nc.scalar.activation(sv[:, 0:96, 0],
                                                 ps[hf][:, 0:96],
                                                 AF.Sigmoid)
                            if not SCO_FUSE:
                                nc.scalar.activation(sv[:, 96:128, 0],
                                                     ps[hf][:, 96:128],
                                                     AF.Sigmoid)
                            sigs[hf] = s_
                        # phase 2: per-half u + c-scan + sigma(4x) + h
                        for hf in (L, R):
                            s_ = sigs[hf]
                            sv = s_[:].rearrange("p (j t2) -> p j t2", t2=2)
                            ca = cil[hf][t % 2]
                            cb = cil[hf][(t + 1) % 2]
                            cv_a = ca[:].rearrange("p (j t2) -> p j t2", t2=2)
                            cv = cb[:].rearrange("p (j t2) -> p j t2", t2=2)
                            # u_j -> even cols 2+2j of the read buffer
                            i_u = nc.vector.scalar_tensor_tensor(
                                cv_a[:, 1:33, 0], sv[:, 64:96, 0], -0.5,
                                sv[:, 0:32, 0], OP.add, OP.mult,
                            )
                            # c~_j = sigf_j * c~_j + u_j  (one scan op;
                            # d0 = sig cols 63..126 = [0, f_0, 0, f_1, ...])
                            i_c = nc.vector.tensor_tensor_scan(
                                cb[:, 0:64], s_[:, 63:127], ca[:, 1:65],
                                0.0, OP.mult, OP.add,
                            )
                            _demote(i_c, i_u)
                            if SCO_FUSE:
                                sco = smlp.tile([128, 65], F32,
                                                tag=f"sc{hf}", name=f"sc{hf}")
                                nc.scalar.activation(sco[:], cv[:, 0:65, 1],
                                                     AF.Sigmoid, scale=4.0)
                                if HH_SPLIT:
                                    for q in range(2):
                                        Sq = slice(32 * hf + 16 * q,
                                                   32 * hf + 16 * (q + 1))
                                        nc.vector.scalar_tensor_tensor(
                                            hT_new[:, Sq],
                                            sco[:, 16 * q:16 * (q + 1)], -0.5,
                                            sco[:, 33 + 16 * q:49 + 16 * q],
                                            OP.add, OP.mult,
                                        )
                                else:
                                    nc.vector.scalar_tensor_tensor(
                                        hT_new[:, 32 * hf:32 * (hf + 1)],
                                        sco[:, 0:32], -0.5, sco[:, 33:65],
                                        OP.add, OP.mult,
                                    )
                                continue
                            sc = smlp.tile([128, 32], F32, tag=f"sc{hf}",
                                           name=f"sc{hf}")
                            nc.scalar.activation(sc[:], cv[:, 0:32, 1],
                                                 AF.Sigmoid, scale=4.0)
                            if HH_SPLIT:
                                for q in range(2):
                                    Sq = slice(32 * hf + 16 * q,
                                               32 * hf + 16 * (q + 1))
                                    nc.vector.scalar_tensor_tensor(
                                        hT_new[:, Sq],
                                        sc[:, 16 * q:16 * (q + 1)], -0.5,
                                        sv[:, 96 + 16 * q:96 + 16 * (q + 1), 0],
                                        OP.add, OP.mult,
                                    )
                            else:
                                nc.vector.scalar_tensor_tensor(
                                    hT_new[:, 32 * hf:32 * (hf + 1)], sc[:],
                                    -0.5, sv[:, 96:128, 0], OP.add, OP.mult,
                                )
                        hT = hT_new
                        if C_FRESH:
                            c_cur = c_new
                        continue
                    sigs = {}
                    # sigmoids first (ACT is in-order; lead half first)
                    for hf in (L, R):
                        s_ = sigp.tile([128, 128], F32, tag=f"sig{hf}",
                                       name=f"sig{hf}")
                        # i,f,g first (gates the c chain), o later
                        nc.scalar.activation(s_[:, 0:96], ps[hf][:, 0:96],
                                             AF.Sigmoid)
                        nc.scalar.activation(s_[:, 96:128],
                                             ps[hf][:, 96:128], AF.Sigmoid)
                        sigs[hf] = s_
                        S = slice(32 * hf, 32 * (hf + 1))
                        if TAIL_DVE:
                            # v, u, c engine-chained on DVE; c's RAW deps on
                            # u and v are enforced by DVE program order, so
                            # the sems are demoted to nosync edges.
                            v = smlp.tile([128, 32], F32, tag=f"v{hf}",
                                          name=f"v{hf}")
                            i_v = nc.vector.tensor_mul(
                                v[:], s_[:, 32:64], c_cur[:, S])
                            u = smlp.tile([128, 32], F32, tag=f"u{hf}",
                                          name=f"u{hf}")
                            i_u = nc.vector.scalar_tensor_tensor(
                                u[:], s_[:, 64:96], -0.5, s_[:, 0:32],
                                OP.add, OP.mult,
                            )
                            i_c = nc.vector.scalar_tensor_tensor(
                                c_new[:, S], u[:], 2.0, v[:],
                                OP.mult, OP.add,
                            )
                            _demote(i_c, i_u, i_v)
                        else:
                            # c update for this half (only TensorTensor is
                            # legal on Pool, so at most the v-multiply can
                            # be offloaded there)
                            v_pool = (POOL_V == 2
                                      or (POOL_V == 1 and hf == R)
                                      or (POOL_V == 3 and hf == L))
                            u = smlp.tile([128, 32], F32, tag=f"u{hf}",
                                          name=f"u{hf}")
                            nc.vector.scalar_tensor_tensor(
                                u[:], s_[:, 64:96], -0.5, s_[:, 0:32],
                                OP.add, OP.mult,
                            )
                            v = smlp.tile([128, 32], F32, tag=f"v{hf}",
                                          name=f"v{hf}")
                            veng = nc.gpsimd if v_pool else nc.vector
                            veng.tensor_mul(v[:], s_[:, 32:64], c_cur[:, S])
                            nc.vector.scalar_tensor_tensor(
                                c_new[:, S], u[:], 2.0, v[:],
                                OP.mult, OP.add,
                            )
                    for hf in (L, R):
                        S = slice(32 * hf, 32 * (hf + 1))
                        sc = smlp.tile([128, 32], F32, tag=f"sc{hf}",
                                       name=f"sc{hf}")
                        nc.scalar.activation(sc[:], c_new[:, S], AF.Sigmoid,
                                             scale=2.0)
                        if HH_SPLIT:
                            for q in range(2):
                                Sq = slice(32 * hf + 16 * q,
                                           32 * hf + 16 * (q + 1))
                                nc.vector.scalar_tensor_tensor(
                                    hT_new[:, Sq], sc[:, 16 * q:16 * (q + 1)],
                                    -0.5, sigs[hf][:, 96 + 16 * q:96 + 16 * (q + 1)],
                                    OP.add, OP.mult,
                                )
                        else:
                            nc.vector.scalar_tensor_tensor(
                                hT_new[:, S], sc[:], -0.5, sigs[hf][:, 96:128],
                                OP.add, OP.mult,
                            )
                    hT = hT_new
                    if C_FRESH:
                        c_cur = c_new
                while early_emitted < len(early_q):
                    early_q[early_emitted]()
                    early_emitted += 1
                for seg in conv_segs:
                    th = seg[0]
                    while seg[3] < len(th):
                        th[seg[3]]()
                        seg[3] += 1

            # ================= Head ========================================
            with (
                tc.tile_pool(name="hd", bufs=1) as hd,
                tc.tile_pool(name="hps", bufs=1, space="PSUM") as hpsp,
            ):
                hps = hpsp.tile([64, BL], F32, tag="hps")
                nc.tensor.matmul(hps[:], b1_sb[:], ones_sb[:],
                                 start=True, stop=False)
                for k in range(4):
                    nc.tensor.matmul(
                        hps[:], w1T_sb[k][:], hT[:, BL * k:BL * (k + 1)],
                        start=False, stop=(k == 3),
                    )
                hid = hd.tile([64, BL], BF16, tag="hid")
                nc.scalar.activation(hid[:], hps[:], AF.Relu)
                lps = hpsp.tile([2, BL], F32, tag="lps")
                nc.tensor.matmul(lps[:], b2_sb[:], ones_sb[:],
                                 start=True, stop=False)
                nc.tensor.matmul(lps[:], w2T_sb[:], hid[:],
                                 start=False, stop=True)
                outt = hd.tile([2, BL], F32, tag="outt")
                nc.vector.tensor_copy(outt[:], lps[:])
                nc.sync.dma_start(out[:], outt[:])

    nc.compile()
    return nc


def _prep(inputs):
    x = np.asarray(inputs["x"], np.float32)
    conv_w = np.asarray(inputs["conv_w"], np.float32)
    bn_gamma = np.asarray(inputs["bn_gamma"], np.float32)
    bn_beta = np.asarray(inputs["bn_beta"], np.float32)
    w_ih = np.asarray(inputs["w_ih"], np.float32)
    w_hh = np.asarray(inputs["w_hh"], np.float32)
    b_ih = np.asarray(inputs["b_ih"], np.float32)
    b_hh = np.asarray(inputs["b_hh"], np.float32)
    w1 = np.asarray(inputs["w1"], np.float32)
    b1 = np.asarray(inputs["b1"], np.float32)
    w2 = np.asarray(inputs["w2"], np.float32)
    b2 = np.asarray(inputs["b2"], np.float32)
    bf = ml_dtypes.bfloat16

    # ---- BN batch statistics (host, exact) ----
    xp_ = np.pad(x, ((0, 0), (0, 0), (2, 2)))
    Xt = np.ascontiguousarray(xp_.transpose(1, 0, 2))  # [C, B, T+4]
    acc = np.zeros((C, B, T), np.float32)
    for k in range(5):
        acc += np.tensordot(conv_w[:, :, k], Xt[:, :, k:k + T], axes=(1, 0))
    mean = acc.mean(axis=(1, 2), dtype=np.float64)
    var = (acc.astype(np.float64) ** 2).mean(axis=(1, 2)) - mean ** 2
    bn_a = (bn_gamma.astype(np.float64) / np.sqrt(var + EPS))
    bn_b = bn_beta.astype(np.float64) - mean * bn_a
    bn_ab = np.stack([bn_a, bn_b], axis=1).astype(np.float32)  # [C, 2]

    # ---- gate permutation: [i | f | o | g] with g rows scaled x2 ----
    perm = np.r_[0:512, 512:1024, 1536:2048, 1024:1536]
    rs = np.ones((G, 1), np.float32)
    rs[1536:2048] = 2.0

    w_ih_p = w_ih[perm] * rs                       # [G, C]
    w_hh_p = w_hh[perm] * rs * 2.0                 # [G, H]
    bias_p = ((b_ih + b_hh)[perm] * rs[:, 0])      # [G]

    wihT = np.ascontiguousarray(w_ih_p.T.reshape(2, 128, G)).astype(bf)
    whhT = np.ascontiguousarray(w_hh_p.T.reshape(4, 128, G)).astype(bf)

    bias4 = bias_p.reshape(4, 4, 128)[[0, 1, 3, 2]]  # col order [i,f,g,o]
    biasT = np.stack([
        bias4[:, 0:2, :].reshape(8, 128),
        bias4[:, 2:4, :].reshape(8, 128),
    ]).astype(bf)                                   # [half, j=col*2+s2, gp]
    maskb = np.zeros((8, 128), np.float32)
    for j in range(8):
        maskb[j, 16 * j:16 * (j + 1)] = 1.0
    maskb = maskb.astype(bf)
    # single-tile variants: j = 8*half + GT2COL[gt]*2 + s2 -> 16-col block
    biasTf = np.zeros((16, 128), np.float32)
    for j in range(16):
        h_, r = j // 8, j % 8
        gt = GT2COL.index(r // 2)
        m = gt * 4 + 2 * h_ + (r % 2)
        biasTf[j] = bias_p[128 * m:128 * (m + 1)]
    biasTf = biasTf.astype(bf)
    maskf = np.zeros((16, 256), np.float32)
    for j in range(16):
        maskf[j, 16 * j:16 * (j + 1)] = 1.0
    maskf = maskf.astype(bf)

    cdt = bf if CONV_BF16 else np.float32
    cw = np.zeros((10, 128, 256), np.float32)
    for k in range(5):
        for cc in range(2):
            cw[k * 2 + cc] = conv_w[:, 128 * cc:128 * (cc + 1), k].T
    if EVICT_DVE:
        # fold the BN scale into the conv weights (per output channel =
        # last axis of cw); eviction then only needs (+bias, relu)
        cw = cw * bn_a.astype(np.float32)[None, None, :]
    cw = cw.astype(cdt)

    w1T = np.ascontiguousarray((2.0 * w1).T.reshape(4, 128, 64)).astype(bf)
    w2T = np.ascontiguousarray(w2.T).astype(bf)

    common = dict(
        cw=cw,
        bn_ab=bn_ab,
        wihT=wihT,
        whhT=whhT,
        biasT=biasT,
        maskb=maskb,
        biasTf=biasTf,
        maskf=maskf,
        w1T=w1T,
        b1r=b1.reshape(1, 64).astype(bf),
        w2T=w2T,
        b2r=b2.reshape(1, 2).astype(bf),
        ones1=np.ones((1, 16), bf),
    )
    in_maps = []
    x_c = x.astype(cdt)
    for core in range(NCORES):
        m = dict(common)
        m["x_l"] = np.ascontiguousarray(x_c[BL * core:BL * (core + 1)])
        in_maps.append(m)
    return in_maps


def kernel(**inputs) -> np.ndarray:
    if "nc" not in _cache:
        _cache["nc"] = _build()
    nc = _cache["nc"]
    in_maps = _prep(inputs)
    res = run_bass_kernel_spmd(nc, in_maps, list(range(NCORES)))
    _cache["last_res"] = res
    return np.concatenate([res.results[c]["out"].T for c in range(NCORES)],
                          axis=0).astype(np.float32)



# revision 49
# speedup vs baseline: 1.0013x; 1.0001x over previous
"""CNN-LSTM decoder Trainium2 kernel (8 NeuronCores, data-parallel over batch).

Model (see reference): Conv1d(256->256,k=5,p=2) -> BatchNorm1d(train stats) ->
ReLU -> LSTM(256->512, T=1024) -> Linear(512->64) -> ReLU -> Linear(64->2).

Distribution: batch 128 split 16/core across 8 cores.  Per core:

  Conv:    time-block tiled ([128, w+4] x slices, N=w bf16 matmuls,
           BN+ReLU fused into the ACT eviction straight into a persistent
           SBUF buffer featsT[cc][ci, t, b] in bf16).  Only the first 32
           output cols are computed before the scan starts; the rest is
           emitted into the scan loop against per-piece consumption
           deadlines and hidden in engine idle gaps by the Tile priority
           scheduler.  BN batch stats are computed host-side.

  Scan:    fully transposed layout.  Gates live as gatesT [128 gate-dims,
           16 batch] PSUM tiles, two halves (H-slices 0-1 / 2-3, one PSUM
           bank each); psum cols = [i|f|g|o]*32 + s2*16 + b.  Per step:
             - 2 bias matmuls (K=8 mask trick) open each half's accum group
             - 32 input-projection matmuls (wihT chunks @ featsT[:, t, :]),
               emitted one step ahead so they fill PE idle time
             - 64 recurrent matmuls (whhT chunks @ hT k-slices), N=16 bf16
           Tail per half (TAIL_SCAN): one sigmoid over i/f/g writing to
           EVEN columns of the sig tile (odd columns preset to zero; g rows
           pre-scaled x2 so tanh(x) = 2*sigmoid(2x)-1), sigmoid over o,
           then u = (sig2g-.5)*sigi into the even cols of an interleaved
           [., c~, u, c~, u, ...] buffer and the whole c-update
           c~_j = sigf_j*c~_j + u_j as ONE DVE tensor_tensor_scan
           (2-element cells [reset, apply]; d0 = [0|sigf] interleaved via
           the preset zeros, ping-pong buffers per step parity).  c~ = c/2,
           so sigma(2c) = sigma(4c~).  h = (sig(4c~)-.5)*sigo is written
           as h/2 in bf16, the exact moving operand of the next step's
           recurrent matmuls.  Same-engine RAW deps in the tail are demoted
           to nosync edges (_demote): in-order engines make the semaphores
           redundant, removing their latency from the recurrence chain.
           The half whose tail finished late leads the next step
           (alternating), keeping both ACT chains near-saturated.  The
           cycle is bound by the dense ACT sequence (2 sigma96 + 2 sigma_o
           + 2 sigma(4c~)) plus the psum-stop coupling.

  Head:    transposed too: hid [64, 16] = relu(w1 @ h + b1); out [2, 16].

All host-side weight massaging (permutation, scaling, transposes, BN stats)
happens in kernel() below; the device kernel is compiled once per process.
"""

import sys

sys.path.insert(0, "/opt/trn_rl_repo")

import ml_dtypes
import numpy as np

import concourse.bass as bass
import concourse.tile as tile
from concourse import bacc, mybir
from concourse.bass_utils import run_bass_kernel_spmd

F32 = mybir.dt.float32
F32R = mybir.dt.float32r
BF16 = mybir.dt.bfloat16
AF = mybir.ActivationFunctionType
OP = mybir.AluOpType

B, C, T, H = 128, 256, 1024, 512
G = 4 * H  # 2048
NCORES = 8
BL = B // NCORES  # 16 examples per core
EPS = 1e-5

# gate chunk m = gt*4 + s (gt in [i,f,o,g], s = H-slice 0..3)
# psum half = s // 2; col within half = gt*32 + (s % 2)*16 + b
# within each half, o-gate chunks (gt==2 in perm order) go last so the
# ifg sigmoid is not gated on the o-column matmuls
A_CHUNKS = ([m for m in range(16) if (m % 4) < 2 and m // 4 != 2]
            + [m for m in range(16) if (m % 4) < 2 and m // 4 == 2])
B_CHUNKS = ([m for m in range(16) if (m % 4) >= 2 and m // 4 != 2]
            + [m for m in range(16) if (m % 4) >= 2 and m // 4 == 2])

PSUM_PAD = True
ALT_LEAD = True
HH_SPLIT = True
C_FRESH = False
POOL_V = 1
KORD = 0
SGROUP = 0
EVICT_DVE = False
CPS_BUFS = 3
DL_MARGIN = 0
HT_BUFS = 2
SIG_BUFS = 2
PS_BUFS = 2
PRE0 = 32
SIG_SPLIT = 2
CONV_STREAM = True
CONV_BF16 = True
DEMOTE = True   # demote same-engine data deps to nosync (engine order
                # guarantees RAW on HW: in-order engines drain the pipe
                # between instructions)
TAIL_DVE = True   # (pre-scan fallback) v on DVE ahead of u/c, engine-chained
TAIL_SCAN = True  # c-update as one tensor_tensor_scan over the interleaved
                  # [c~|u] ping-pong buffers (c~ = c/2)
SIG_MERGE = False   # (tried) merge sigma_ifg+sigma_o: breaks phase stagger
PS_ONE = False      # (tried) both halves in one psum bank: re-phases badly
SIG_STOPDEP = False
CONV_SEGS = ((128, 192), (320, 192), (512, 192), (704, 192), (896, 128))
EARLY_DL = 90
EARLY_T0 = 16   # don't emit early conv pieces before this step
XP_BUFS = 6
DMA_SPLIT = True  # per-slice scan-weight DMAs so step 0 starts earlier
SCO_FUSE = False    # (tried) fuse sigma_o into sigma(4c~): o-copy lands on
                    # the DVE critical chain

_cache = {}


def _demote(cons, *prods):
    """Demote cons's sync deps on prods to nosync deps.

    Only valid when cons and every prod run on the same engine: the
    scheduler still enforces ordering, and in-order engine execution
    makes the RAW safe without a semaphore."""
    if not DEMOTE:
        return
    ci = cons.ins
    sync = ci.sync_dependency_set_copy()
    for p in prods:
        pi = p.ins
        assert pi.engine == ci.engine, (pi.engine, ci.engine)
        if pi.name in sync:
            ci.remove_dependency(pi.name)
            ci.add_dependency(
                pi.name, mybir.DependencyInfo(sync=False, no_sync=True)
            )


GT2COL = (0, 1, 3, 2)  # perm order [i,f,o,g] -> col blocks [i,f,g,o]


def _mm_dest(psA, psB, m):
    gt, s = m // 4, m % 4
    if PS_ONE:
        c0 = 128 * (s // 2) + GT2COL[gt] * 32 + (s % 2) * 16
        return psA, c0
    ps = psA if s < 2 else psB
    c0 = GT2COL[gt] * 32 + (s % 2) * 16
    return ps, c0


def _build(nT=T, skip_conv=False):
    CDT = BF16 if CONV_BF16 else F32R
    nc = bacc.Bacc("TRN2", target_bir_lowering=False, debug=False,
                   num_devices=NCORES)

    x_l = nc.dram_tensor("x_l", [BL, C, T], CDT, kind="ExternalInput").ap()
    cw = nc.dram_tensor("cw", [10, 128, 256], CDT, kind="ExternalInput").ap()
    bn_ab = nc.dram_tensor("bn_ab", [C, 2], F32, kind="ExternalInput").ap()
    wihT = nc.dram_tensor("wihT", [2, 128, G], BF16, kind="ExternalInput").ap()
    whhT = nc.dram_tensor("whhT", [4, 128, G], BF16, kind="ExternalInput").ap()
    biasT = nc.dram_tensor("biasT", [2, 8, 128], BF16, kind="ExternalInput").ap()
    maskb = nc.dram_tensor("maskb", [8, 128], BF16, kind="ExternalInput").ap()
    biasTf = nc.dram_tensor("biasTf", [16, 128], BF16, kind="ExternalInput").ap()
    maskf = nc.dram_tensor("maskf", [16, 256], BF16, kind="ExternalInput").ap()
    w1T = nc.dram_tensor("w1T", [4, 128, 64], BF16, kind="ExternalInput").ap()
    b1r = nc.dram_tensor("b1r", [1, 64], BF16, kind="ExternalInput").ap()
    w2T = nc.dram_tensor("w2T", [64, 2], BF16, kind="ExternalInput").ap()
    b2r = nc.dram_tensor("b2r", [1, 2], BF16, kind="ExternalInput").ap()
    ones1 = nc.dram_tensor("ones1", [1, 16], BF16, kind="ExternalInput").ap()

    out = nc.dram_tensor("out", [2, BL], F32, kind="ExternalOutput").ap()

    if SCO_FUSE:
        # o-gate chunks first: their psum cols finish early so the DVE
        # o-copies run during the whh block, fully hidden
        A_CH = ([m for m in range(16) if (m % 4) < 2 and m // 4 == 2]
                + [m for m in range(16) if (m % 4) < 2 and m // 4 != 2])
        B_CH = ([m for m in range(16) if (m % 4) >= 2 and m // 4 == 2]
                + [m for m in range(16) if (m % 4) >= 2 and m // 4 != 2])
    else:
        A_CH, B_CH = A_CHUNKS, B_CHUNKS

    with tile.TileContext(nc) as tc:
        with (
            tc.tile_pool(name="const", bufs=1) as const,
            tc.tile_pool(name="state", bufs=1) as state,
        ):
            # ---- persistent constants in SBUF ----
            cwb = const.tile([128, 10 * 256], CDT, tag="cwb")
            cw_sb = [cwb[:, 256 * i:256 * (i + 1)] for i in range(10)]
            nc.sync.dma_start(
                cwb[:].rearrange("p (n c) -> p n c", n=10),
                cw[0:10].transpose([1, 0, 2]),
            )
            bn_sb = [const.tile([128, 2], F32, name=f"bn{i}", tag=f"bn{i}")
                     for i in range(2)]
            for i in range(2):
                nc.sync.dma_start(bn_sb[i][:], bn_ab[128 * i:128 * (i + 1), :])
            # (DMAs for the tensors below are emitted after the conv
            # prelude so the prelude's x slices go first in the DMA queue;
            # they only have to arrive before scan step 0's matmuls.)
            wihb = const.tile([128, 2 * G], BF16, tag="wihb")
            wih_sb = [wihb[:, G * i:G * (i + 1)] for i in range(2)]
            whhb = const.tile([128, 4 * G], BF16, tag="whhb")
            whh_sb = [whhb[:, G * i:G * (i + 1)] for i in range(4)]
            biasT_sb = [const.tile([8, 128], BF16, name=f"biasT{i}",
                                   tag=f"biasT{i}") for i in range(2)]
            mask_sb = const.tile([8, 128], BF16, tag="mask_sb")
            biasTf_sb = const.tile([16, 128], BF16, tag="biasTf_sb")
            maskf_sb = const.tile([16, 256], BF16, tag="maskf_sb")
            w1T_sb = [const.tile([128, 64], BF16, name=f"w1T{i}", tag=f"w1T{i}")
                      for i in range(4)]
            b1_sb = const.tile([1, 64], BF16, tag="b1_sb")
            w2T_sb = const.tile([64, 2], BF16, tag="w2T_sb")
            b2_sb = const.tile([1, 2], BF16, tag="b2_sb")
            ones_sb = const.tile([1, 16], BF16, tag="ones_sb")

            def load_scan_weights():
                if DMA_SPLIT:
                    # bias/ih first (step 0's psum opens need them), then
                    # whh per k-slice so step 0's k0 matmuls start before
                    # the whole 16KB/partition transfer lands
                    for i in range(2):
                        nc.sync.dma_start(wihb[:, G * i:G * (i + 1)],
                                          wihT[i])
                    for i in range(4):
                        nc.sync.dma_start(whhb[:, G * i:G * (i + 1)],
                                          whhT[i])
                else:
                    nc.sync.dma_start(
                        wihb[:].rearrange("p (n g) -> p n g", n=2),
                        wihT[0:2].transpose([1, 0, 2]),
                    )
                    nc.sync.dma_start(
                        whhb[:].rearrange("p (n g) -> p n g", n=4),
                        whhT[0:4].transpose([1, 0, 2]),
                    )
                if PS_ONE:
                    nc.sync.dma_start(biasTf_sb[:], biasTf[:])
                    nc.sync.dma_start(maskf_sb[:], maskf[:])
                else:
                    for i in range(2):
                        nc.sync.dma_start(biasT_sb[i][:], biasT[i])
                    nc.sync.dma_start(mask_sb[:], maskb[:])
                for i in range(4):
                    nc.sync.dma_start(w1T_sb[i][:], w1T[i])
                nc.sync.dma_start(b1_sb[:], b1r[:])
                nc.sync.dma_start(w2T_sb[:], w2T[:])
                nc.sync.dma_start(b2_sb[:], b2r[:])
                nc.sync.dma_start(ones_sb[:], ones1[:])

            # ---- persistent activations / state ----
            featsT = [state.tile([128, T, BL], BF16, name=f"featsT{i}",
                                 tag=f"featsT{i}") for i in range(2)]
            c_st = state.tile([128, 4 * BL], F32, tag="c_st")
            nc.vector.memset(c_st[:], 0.0)
            # TAIL_SCAN: per-half interleaved [., c~_0, u_0, c~_1, u_1, ...]
            # buffer (c~ = c/2 at odd cols 1+2j, u at even cols 2+2j).  The
            # c-recurrence c~_j = sigmf_j * c~_j + u_j is ONE DVE
            # tensor_tensor_scan per half: 2-element cells [reset, apply]
            # with d0 = [0, sigmf] (zeros preset at odd cols of the sig
            # tile), d1 = this buffer.
            # ping-pong per step parity: scan(t) reads buf[t%2], writes
            # buf[(t+1)%2]; the o-copy also targets the write buffer
            cil = [[state.tile([128, 132], F32, name=f"cil{h_}{p_}",
                               tag=f"cil{h_}{p_}") for p_ in range(2)]
                   for h_ in range(2)]
            for h_ in range(2):
                for p_ in range(2):
                    nc.vector.memset(cil[h_][p_][:], 0.0)
            # C_FRESH: c lives in a rotating pool instead (see scan loop)
            if skip_conv:
                for i in range(2):
                    nc.vector.memset(featsT[i][:].bitcast(F32), 0.0)

            # ===== Conv (tq-streamed into the scan) + Scan =================
            # conv tiled as 8 time-blocks of 128; block 0 is a short
            # prelude, blocks 1..7 are emitted into the scan loop at a
            # uniform rate so block j is ready before scan step 128*j.
            # The Tile priority scheduler slots the ops into idle engine
            # gaps (PE ~60%, ACT ~50% idle during the scan).
            def conv_piece_thunks(t0, w):
                # conv for output cols [t0, t0+w) over all examples/channels
                thunks = []
                if skip_conv:
                    return thunks
                xts = {}

                def load_x(ex, t0=t0, w=w):
                    lo = t0 - 2
                    pair = []
                    for cc in range(2):
                        t_ = xp.tile([128, w + 4], CDT, name=f"xs{cc}",
                                     tag=f"xs{cc}")
                        s0, s1 = max(lo, 0), min(lo + w + 4, T)
                        if t0 == 0:
                            nc.vector.memset(t_[:, 0:2].bitcast(F32), 0.0)
                        if t0 + w == T:
                            nc.vector.memset(
                                t_[:, w + 2:w + 4].bitcast(F32), 0.0)
                        nc.sync.dma_start(
                            t_[:, s0 - lo:s1 - lo],
                            x_l[ex, 128 * cc:128 * (cc + 1), s0:s1],
                        )
                        pair.append(t_)
                    xts[ex] = pair

                for ex in range(BL):
                    thunks.append(lambda ex=ex: load_x(ex))
                    for co in range(2):
                        cst = {}

                        def mk_mm(ex, co, cc, k, first, w=w, cst=cst):
                            def mm():
                                if first:
                                    cst["ps"] = cpsp.tile([128, w], F32,
                                                          tag="cps",
                                                          name="cps",
                                                          padded_shape=[128, 512])
                                nc.tensor.matmul(
                                    cst["ps"][:],
                                    cw_sb[k * 2 + cc][:, 128 * co:128 * (co + 1)],
                                    xts[ex][cc][:, k:k + w],
                                    start=first,
                                    stop=(cc == 1 and k == 4),
                                )
                            return mm

                        first = True
                        for cc in range(2):
                            for k in range(5):
                                thunks.append(mk_mm(ex, co, cc, k, first))
                                first = False

                        def evict(ex=ex, co=co, t0=t0, w=w, cst=cst):
                            if EVICT_DVE:
                                # bn scale folded into conv weights host-side
                                nc.vector.tensor_scalar(
                                    featsT[co][:, t0:t0 + w, ex],
                                    cst["ps"][:], bn_sb[co][:, 1:2], 0.0,
                                    OP.add, OP.max,
                                )
                            else:
                                nc.scalar.activation(
                                    featsT[co][:, t0:t0 + w, ex],
                                    cst["ps"][:], AF.Relu,
                                    bias=bn_sb[co][:, 1:2],
                                    scale=bn_sb[co][:, 0:1],
                                )
                        thunks.append(evict)
                return thunks

            # ================= Scan (transposed layout) ====================
            with (
                tc.tile_pool(name="xp", bufs=XP_BUFS) as xp,
                tc.tile_pool(name="cps", bufs=CPS_BUFS, space="PSUM") as cpsp,
                tc.tile_pool(name="hTp", bufs=HT_BUFS) as hTp,
                tc.tile_pool(name="sig", bufs=SIG_BUFS) as sigp,
                tc.tile_pool(name="sml", bufs=SIG_BUFS) as smlp,
                tc.tile_pool(name="psA", bufs=PS_BUFS, space="PSUM") as psAp,
                tc.tile_pool(name="psB", bufs=PS_BUFS, space="PSUM") as psBp,
            ):
                # conv prelude: only the first PRE0 cols must precede
                # step 0; the rest of block 0 streams into the first steps
                # with tight deadlines, blocks 1..7 at a uniform rate.
                # prelude piece with the x loads batched into one wide
                # DMA per cc (32 separate dma_starts would serialize ~18us
                # of SP sequencer time before the first conv matmul)
                if not skip_conv:
                    wp = PRE0 + 4
                    xb = [xp.tile([128, BL * wp], CDT, name=f"xb{cc}",
                                  tag=f"xb{cc}") for cc in range(2)]
                    for cc in range(2):
                        xv = xb[cc][:].rearrange("p (e w) -> p e w", e=BL)
                        nc.vector.memset(xv[:, :, 0:2].bitcast(F32), 0.0)
                        nc.sync.dma_start(
                            xv[:, :, 2:wp],
                            x_l[0:BL, 128 * cc:128 * (cc + 1),
                                0:PRE0 + 2].transpose([1, 0, 2]),
                        )
                    for ex in range(BL):
                        for co in range(2):
                            pps = cpsp.tile([128, PRE0], F32, tag="cps",
                                            name="cps",
                                            padded_shape=[128, 128])
                            first = True
                            for cc in range(2):
                                for k in range(5):
                                    nc.tensor.matmul(
                                        pps[:],
                                        cw_sb[k * 2 + cc][:, 128 * co:128 * (co + 1)],
                                        xb[cc][:, ex * wp + k:ex * wp + k + PRE0],
                                        start=first,
                                        stop=(cc == 1 and k == 4),
                                    )
                                    first = False
                            if EVICT_DVE:
                                nc.vector.tensor_scalar(
                                    featsT[co][:, 0:PRE0, ex],
                                    pps[:], bn_sb[co][:, 1:2], 0.0,
                                    OP.add, OP.max,
                                )
                            else:
                                nc.scalar.activation(
                                    featsT[co][:, 0:PRE0, ex],
                                    pps[:], AF.Relu,
                                    bias=bn_sb[co][:, 1:2],
                                    scale=bn_sb[co][:, 0:1],
                                )
                load_scan_weights()
                early_q = []
                for t0, w0 in ((PRE0, PRE0), (2 * PRE0, 2 * PRE0)):
                    early_q += conv_piece_thunks(t0, w0)
                early_dl = EARLY_DL
                early_emitted = 0
                # wider mid pieces halve the ACT eviction count; each
                # piece must fully evict before the scan consumes its first
                # column, so each gets its own deadline window.
                conv_segs = []
                prev = 0
                for t0, w in CONV_SEGS:
                    s1 = max(1, min(nT, t0 - 6))
                    conv_segs.append(
                        [conv_piece_thunks(t0, w), prev, s1, 0])
                    prev = s1

                hT = hTp.tile([128, 4 * BL], BF16, tag="hT", name="hT")
                nc.vector.memset(hT[:].bitcast(F32), 0.0)
                if C_FRESH:
                    c_cur = hTp.tile([128, 4 * BL], F32, tag="cT", name="cT")
                    nc.vector.memset(c_cur[:], 0.0)
                else:
                    c_cur = c_st
                if TAIL_SCAN and PS_ONE:
                    # preset the sig-pool buffers' odd columns to zero once;
                    # in-loop writers only touch even columns.
                    for _b in range(SIG_BUFS):
                        s0 = sigp.tile([128, 512], F32, tag="sig", name="sig")
                        sv0 = s0[:].rearrange("p (j t) -> p j t", t=2)
                        nc.vector.memset(sv0[:, :, 1], 0.0)
                elif TAIL_SCAN:
                    for _b in range(SIG_BUFS):
                        for hf in range(2):
                            s0 = sigp.tile([128, 256], F32, tag=f"sig{hf}",
                                           name=f"sig{hf}")
                            sv0 = s0[:].rearrange("p (j t) -> p j t", t=2)
                            nc.vector.memset(sv0[:, :, 1], 0.0)

                CH = (A_CH, B_CH)             # chunks per half
                KS = ((0, 1), (2, 3))         # hT k-slices produced per half
                def open_step(t):
                    # allocate this step's gate psums, open the accumulation
                    # groups with the bias matmuls, and emit the input
                    # projection.  Called one step ahead so these (dependency-
                    # free) matmuls sit ahead of the waiting whh matmuls in
                    # PE's in-order queue and fill its idle time.
                    L = (t % 2) if ALT_LEAD else 0
                    R = 1 - L
                    pshape = [128, 512] if PSUM_PAD else None
                    if PS_ONE:
                        pt = psAp.tile([128, 256], F32, tag="ps", name="ps",
                                       padded_shape=pshape)
                        ps = [pt, pt]
                        nc.tensor.matmul(pt[:], biasTf_sb[:], maskf_sb[:],
                                         start=True, stop=False)
                    else:
                        ps = [None, None]
                        ps[L] = (psAp if L == 0 else psBp).tile(
                            [128, 128], F32, tag=f"ps{L}", name=f"ps{L}",
                            padded_shape=pshape)
                        ps[R] = (psAp if R == 0 else psBp).tile(
                            [128, 128], F32, tag=f"ps{R}", name=f"ps{R}",
                            padded_shape=pshape)
                        nc.tensor.matmul(ps[L][:], biasT_sb[L][:], mask_sb[:],
                                         start=True, stop=False)
                        nc.tensor.matmul(ps[R][:], biasT_sb[R][:], mask_sb[:],
                                         start=True, stop=False)
                    for m in CH[L] + CH[R]:
                        p_, c0 = _mm_dest(ps[0], ps[1], m)
                        for cc in range(2):
                            nc.tensor.matmul(
                                p_[:, c0:c0 + BL],
                                wih_sb[cc][:, 128 * m:128 * (m + 1)],
                                featsT[cc][:, t, :],
                                start=False, stop=False,
                            )
                    return ps

                ps_next = open_step(0)
                for t in range(nT):
                    # stream conv emission at a uniform rate
                    etarget = min(len(early_q),
                                  int(len(early_q) * max(0, t + 1 - EARLY_T0)
                                      / (early_dl - EARLY_T0)))
                    while early_emitted < etarget:
                        early_q[early_emitted]()
                        early_emitted += 1
                    for seg in conv_segs:
                        th, s0, s1, done = seg
                        if t < s0 or done >= len(th):
                            continue
                        tgt = min(len(th),
                                  int(len(th) * (t + 1 - s0) / (s1 - s0)))
                        while seg[3] < tgt:
                            th[seg[3]]()
                            seg[3] += 1
                    # lead half L: its tail (and thus hT slices) finish early;
                    # alternate so the late half of step t leads step t+1.
                    L = (t % 2) if ALT_LEAD else 0
                    R = 1 - L
                    ps = ps_next
                    if t + 1 < nT:
                        ps_next = open_step(t + 1)
                    # recurrent term.  K[R] slices were produced by last
                    # step's lead tail (early) - emit them first; the final
                    # k group is gated by last step's trailing hh.  Within
                    # it, close the lead half's psum first.
                    for k in KS[R]:
                        for m in CH[L] + CH[R]:
                            p_, c0 = _mm_dest(ps[0], ps[1], m)
                            nc.tensor.matmul(
                                p_[:, c0:c0 + BL],
                                whh_sb[k][:, 128 * m:128 * (m + 1)],
                                hT[:, BL * k:BL * (k + 1)],
                                start=False, stop=False,
                            )
                    i_stop = None
                    for chunks in (CH[L], CH[R]):
                        for k in KS[L]:
                            for m in chunks:
                                p_, c0 = _mm_dest(ps[0], ps[1], m)
                                last = k == KS[L][-1] and m == chunks[-1]
                                if PS_ONE:
                                    stop_ = last and chunks is CH[R]
                                else:
                                    stop_ = last
                                i_mm = nc.tensor.matmul(
                                    p_[:, c0:c0 + BL],
                                    whh_sb[k][:, 128 * m:128 * (m + 1)],
                                    hT[:, BL * k:BL * (k + 1)],
                                    start=False, stop=stop_,
                                )
                                if stop_:
                                    i_stop = i_mm

                    hT_new = hTp.tile([128, 4 * BL], BF16, tag="hT", name="hT")
                    if C_FRESH:
                        c_new = hTp.tile([128, 4 * BL], F32, tag="cT",
                                         name="cT")
                    else:
                        c_new = c_cur
                    if TAIL_SCAN and PS_ONE:
                        # one sig tile for both halves: half hf at cols
                        # [256*hf, 256*hf+256), gates at even cols (zeros
                        # preset at odd cols).
                        sg = sigp.tile([128, 512], F32, tag="sig", name="sig")
                        sgv = sg[:].rearrange("p (h j t) -> p h j t",
                                              h=2, t=2)
                        i_sL = nc.scalar.activation(
                            sgv[:, L, 0:96, 0], ps[L][:, 128 * L:128 * L + 96],
                            AF.Sigmoid)
                        i_sR = nc.scalar.activation(
                            sgv[:, R, 0:96, 0], ps[R][:, 128 * R:128 * R + 96],
                            AF.Sigmoid)
                        psv = ps[0][:].rearrange("p (h c) -> p h c", h=2)
                        i_so = nc.scalar.activation(
                            sgv[:, :, 96:128, 0], psv[:, :, 96:128],
                            AF.Sigmoid)
                        # sub-range gating only (same PE-write/ACT-read bank
                        # overlap the 2-bank baseline already runs with on
                        # HW).  SIG_STOPDEP restores strict full-group
                        # gating if needed.
                        if SIG_STOPDEP and i_stop is not None:
                            i_sL.ins.add_dependency(
                                i_stop.ins.name,
                                mybir.DependencyInfo(sync=True, no_sync=False))
                            i_sR.ins.add_dependency(
                                i_sL.ins.name,
                                mybir.DependencyInfo(sync=False, no_sync=True))
                            i_so.ins.add_dependency(
                                i_sR.ins.name,
                                mybir.DependencyInfo(sync=False, no_sync=True))
                        sigs = {L: None, R: None}
                        for hf in (L, R):
                            sv = sgv[:, hf]
                            ci_ = cil[hf]
                            cv = ci_[:].rearrange("p (j t) -> p j t", t=2)
                            i_u = nc.vector.scalar_tensor_tensor(
                                cv[:, 1:33, 0], sv[:, 64:96, 0], -0.5,
                                sv[:, 0:32, 0], OP.add, OP.mult,
                            )
                            i_c = nc.vector.tensor_tensor_scan(
                                ci_[:, 0:64], sg[:, 256 * hf + 63:256 * hf + 127],
                                ci_[:, 1:65], 0.0, OP.mult, OP.add,
                            )
                            _demote(i_c, i_u)
                            sc = smlp.tile([128, 32], F32, tag=f"sc{hf}",
                                           name=f"sc{hf}")
                            nc.scalar.activation(sc[:], cv[:, 0:32, 1],
                                                 AF.Sigmoid, scale=4.0)
                            if HH_SPLIT:
                                for q in range(2):
                                    Sq = slice(32 * hf + 16 * q,
                                               32 * hf + 16 * (q + 1))
                                    nc.vector.scalar_tensor_tensor(
                                        hT_new[:, Sq],
                                        sc[:, 16 * q:16 * (q + 1)], -0.5,
                                        sv[:, 96 + 16 * q:96 + 16 * (q + 1), 0],
                                        OP.add, OP.mult,
                                    )
                            else:
                                nc.vector.scalar_tensor_tensor(
                                    hT_new[:, 32 * hf:32 * (hf + 1)], sc[:],
                                    -0.5, sv[:, 96:128, 0], OP.add, OP.mult,
                                )
                        hT = hT_new
                        continue
                    if TAIL_SCAN:
                        if SCO_FUSE:
                            # o-gates (/4) -> odd cols 67..129 of the WRITE
                            # buffer.  Emitted BEFORE the sigmoids so the
                            # DVE queue reaches them before any
                            # sigma-dependent wait; with o-gate chunks
                            # ordered first their psum cols are ready
                            # during the whh block.
                            for hf in (L, R):
                                cvh = cil[hf][(t + 1) % 2][:].rearrange(
                                    "p (j t2) -> p j t2", t2=2)
                                nc.vector.tensor_scalar_mul(
                                    cvh[:, 33:65, 1], ps[hf][:, 96:128], 0.25)
                        sigs = {}
                        # phase 1: ifg sigmoids (ACT in-order; lead first).
                        # sig layout: gates at even cols (zeros preset at
                        # odd cols).
                        for hf in (L, R):
                            s_ = sigp.tile([128, 256], F32, tag=f"sig{hf}",
                                           name=f"sig{hf}")
                            sv = s_[:].rearrange("p (j t) -> p j t", t=2)
                            nc.scalar.activation(sv[:, 0:96, 0],
                                                 ps[hf][:, 0:96],
                                                 AF.Sigmoid)
                            if not SCO_FUSE:
                                nc.scalar.activation(sv[:, 96:128, 0],
                                                     ps[hf][:, 96:128],
                                                     AF.Sigmoid)
                            sigs[hf] = s_
                        # phase 2: per-half u + c-scan + sigma(4x) + h
                        for hf in (L, R):
                            s_ = sigs[hf]
                            sv = s_[:].rearrange("p (j t2) -> p j t2", t2=2)
                            ca = cil[hf][t % 2]
                            cb = cil[hf][(t + 1) % 2]
                            cv_a = ca[:].rearrange("p (j t2) -> p j t2", t2=2)
                            cv = cb[:].rearrange("p (j t2) -> p j t2", t2=2)
                            # u_j -> even cols 2+2j of the read buffer
                            i_u = nc.vector.scalar_tensor_tensor(
                                cv_a[:, 1:33, 0], sv[:, 64:96, 0], -0.5,
                                sv[:, 0:32, 0], OP.add, OP.mult,
                            )
                            # c~_j = sigf_j * c~_j + u_j  (one scan op;
                            # d0 = sig cols 63..126 = [0, f_0, 0, f_1, ...])
                            i_c = nc.vector.tensor_tensor_scan(
                                cb[:, 0:64], s_[:, 63:127], ca[:, 1:65],
                                0.0, OP.mult, OP.add,
                            )
                            _demote(i_c, i_u)
                            if SCO_FUSE:
                                sco = smlp.tile([128, 65], F32,
                                                tag=f"sc{hf}", name=f"sc{hf}")
                                nc.scalar.activation(sco[:], cv[:, 0:65, 1],
                                                     AF.Sigmoid, scale=4.0)
                                if HH_SPLIT:
                                    for q in range(2):
                                        Sq = slice(32 * hf + 16 * q,
                                                   32 * hf + 16 * (q + 1))
                                        nc.vector.scalar_tensor_tensor(
                                            hT_new[:, Sq],
                                            sco[:, 16 * q:16 * (q + 1)], -0.5,
                                            sco[:, 33 + 16 * q:49 + 16 * q],
                                            OP.add, OP.mult,
                                        )
                                else:
                                    nc.vector.scalar_tensor_tensor(
                                        hT_new[:, 32 * hf:32 * (hf + 1)],
                                        sco[:, 0:32], -0.5, sco[:, 33:65],
                                        OP.add, OP.mult,
                                    )
                                continue
                            sc = smlp.tile([128, 32], F32, tag=f"sc{hf}",
                                           name=f"sc{hf}")
                            nc.scalar.activation(sc[:], cv[:, 0:32, 1],
                                                 AF.Sigmoid, scale=4.0)
                            if HH_SPLIT:
                                for q in range(2):
                                    Sq = slice(32 * hf + 16 * q,
                                               32 * hf + 16 * (q + 1))
                                    nc.vector.scalar_tensor_tensor(
                                        hT_new[:, Sq],
                                        sc[:, 16 * q:16 * (q + 1)], -0.5,
                                        sv[:, 96 + 16 * q:96 + 16 * (q + 1), 0],
                                        OP.add, OP.mult,
                                    )
                            else:
                                nc.vector.scalar_tensor_tensor(
                                    hT_new[:, 32 * hf:32 * (hf + 1)], sc[:],
                                    -0.5, sv[:, 96:128, 0], OP.add, OP.mult,
                                )
                        hT = hT_new
                        if C_FRESH:
                            c_cur = c_new
                        continue
                    sigs = {}
                    # sigmoids first (ACT is in-order; lead half first)
                    for hf in (L, R):
                        s_ = sigp.tile([128, 128], F32, tag=f"sig{hf}",
                                       name=f"sig{hf}")
                        # i,f,g first (gates the c chain), o later
                        nc.scalar.activation(s_[:, 0:96], ps[hf][:, 0:96],
                                             AF.Sigmoid)
                        nc.scalar.activation(s_[:, 96:128],
                                             ps[hf][:, 96:128], AF.Sigmoid)
                        sigs[hf] = s_
                        S = slice(32 * hf, 32 * (hf + 1))
                        if TAIL_DVE:
                            # v, u, c engine-chained on DVE; c's RAW deps on
                            # u and v are enforced by DVE program order, so
                            # the sems are demoted to nosync edges.
                            v = smlp.tile([128, 32], F32, tag=f"v{hf}",
                                          name=f"v{hf}")
                            i_v = nc.vector.tensor_mul(
                                v[:], s_[:, 32:64], c_cur[:, S])
                            u = smlp.tile([128, 32], F32, tag=f"u{hf}",
                                          name=f"u{hf}")
                            i_u = nc.vector.scalar_tensor_tensor(
                                u[:], s_[:, 64:96], -0.5, s_[:, 0:32],
                                OP.add, OP.mult,
                            )
                            i_c = nc.vector.scalar_tensor_tensor(
                                c_new[:, S], u[:], 2.0, v[:],
                                OP.mult, OP.add,
                            )
                            _demote(i_c, i_u, i_v)
                        else:
                            # c update for this half (only TensorTensor is
                            # legal on Pool, so at most the v-multiply can
                            # be offloaded there)
                            v_pool = (POOL_V == 2
                                      or (POOL_V == 1 and hf == R)
                                      or (POOL_V == 3 and hf == L))
                            u = smlp.tile([128, 32], F32, tag=f"u{hf}",
                                          name=f"u{hf}")
                            nc.vector.scalar_tensor_tensor(
                                u[:], s_[:, 64:96], -0.5, s_[:, 0:32],
                                OP.add, OP.mult,
                            )
                            v = smlp.tile([128, 32], F32, tag=f"v{hf}",
                                          name=f"v{hf}")
                            veng = nc.gpsimd if v_pool else nc.vector
                            veng.tensor_mul(v[:], s_[:, 32:64], c_cur[:, S])
                            nc.vector.scalar_tensor_tensor(
                                c_new[:, S], u[:], 2.0, v[:],
                                OP.mult, OP.add,
                            )
                    for hf in (L, R):
                        S = slice(32 * hf, 32 * (hf + 1))
                        sc = smlp.tile([128, 32], F32, tag=f"sc{hf}",
                                       name=f"sc{hf}")
                        nc.scalar.activation(sc[:], c_new[:, S], AF.Sigmoid,
                                             scale=2.0)
                        if HH_SPLIT:
                            for q in range(2):
                                Sq = slice(32 * hf + 16 * q,
                                           32 * hf + 16 * (q + 1))
                                nc.vector.scalar_tensor_tensor(
                                    hT_new[:, Sq], sc[:, 16 * q:16 * (q + 1)],
                                    -0.5, sigs[hf][:, 96 + 16 * q:96 + 16 * (q + 1)],
                                    OP.add, OP.mult,
                                )
                        else:
                            nc.vector.scalar_tensor_tensor(
                                hT_new[:, S], sc[:], -0.5, sigs[hf][:, 96:128],
                                OP.add, OP.mult,
                            )
                    hT = hT_new
                    if C_FRESH:
                        c_cur = c_new
                while early_emitted < len(early_q):
                    early_q[early_emitted]()
                    early_emitted += 1
                for seg in conv_segs:
                    th = seg[0]
                    while seg[3] < len(th):
                        th[seg[3]]()
                        seg[3] += 1

            # ================= Head ========================================
            with (
                tc.tile_pool(name="hd", bufs=1) as hd,
                tc.tile_pool(name="hps", bufs=1, space="PSUM") as hpsp,
            ):
                hps = hpsp.tile([64, BL], F32, tag="hps")
                nc.tensor.matmul(hps[:], b1_sb[:], ones_sb[:],
                                 start=True, stop=False)
                for k in range(4):
                    nc.tensor.matmul(
                        hps[:], w1T_sb[k][:], hT[:, BL * k:BL * (k + 1)],
                        start=False, stop=(k == 3),
                    )
                hid = hd.tile([64, BL], BF16, tag="hid")
                nc.scalar.activation(hid[:], hps[:], AF.Relu)
                lps = hpsp.tile([2, BL], F32, tag="lps")
                nc.tensor.matmul(lps[:], b2_sb[:], ones_sb[:],
                                 start=True, stop=False)
                nc.tensor.matmul(lps[:], w2T_sb[:], hid[:],
                                 start=False, stop=True)
                outt = hd.tile([2, BL], F32, tag="outt")
                nc.vector.tensor_copy(outt[:], lps[:])
                nc.sync.dma_start(out[:], outt[:])

    nc.compile()
    return nc


def _prep(inputs):
    x = np.asarray(inputs["x"], np.float32)
    conv_w = np.asarray(inputs["conv_w"], np.float32)
    bn_gamma = np.asarray(inputs["bn_gamma"], np.float32)
    bn_beta = np.asarray(inputs["bn_beta"], np.float32)
    w_ih = np.asarray(inputs["w_ih"], np.float32)
    w_hh = np.asarray(inputs["w_hh"], np.float32)
    b_ih = np.asarray(inputs["b_ih"], np.float32)
    b_hh = np.asarray(inputs["b_hh"], np.float32)
    w1 = np.asarray(inputs["w1"], np.float32)
    b1 = np.asarray(inputs["b1"], np.float32)
    w2 = np.asarray(inputs["w2"], np.float32)
    b2 = np.asarray(inputs["b2"], np.float32)
    bf = ml_dtypes.bfloat16

    # ---- BN batch statistics (host, exact) ----
    xp_ = np.pad(x, ((0, 0), (0, 0), (2, 2)))
    Xt = np.ascontiguousarray(xp_.transpose(1, 0, 2))  # [C, B, T+4]
    acc = np.zeros((C, B, T), np.float32)
    for k in range(5):
        acc += np.tensordot(conv_w[:, :, k], Xt[:, :, k:k + T], axes=(1, 0))
    mean = acc.mean(axis=(1, 2), dtype=np.float64)
    var = (acc.astype(np.float64) ** 2).mean(axis=(1, 2)) - mean ** 2
    bn_a = (bn_gamma.astype(np.float64) / np.sqrt(var + EPS))
    bn_b = bn_beta.astype(np.float64) - mean * bn_a
    bn_ab = np.stack([bn_a, bn_b], axis=1).astype(np.float32)  # [C, 2]

    # ---- gate permutation: [i | f | o | g] with g rows scaled x2 ----
    perm = np.r_[0:512, 512:1024, 1536:2048, 1024:1536]
    rs = np.ones((G, 1), np.float32)
    rs[1536:2048] = 2.0

    w_ih_p = w_ih[perm] * rs                       # [G, C]
    w_hh_p = w_hh[perm] * rs * 2.0                 # [G, H]
    bias_p = ((b_ih + b_hh)[perm] * rs[:, 0])      # [G]

    wihT = np.ascontiguousarray(w_ih_p.T.reshape(2, 128, G)).astype(bf)
    whhT = np.ascontiguousarray(w_hh_p.T.reshape(4, 128, G)).astype(bf)

    bias4 = bias_p.reshape(4, 4, 128)[[0, 1, 3, 2]]  # col order [i,f,g,o]
    biasT = np.stack([
        bias4[:, 0:2, :].reshape(8, 128),
        bias4[:, 2:4, :].reshape(8, 128),
    ]).astype(bf)                                   # [half, j=col*2+s2, gp]
    maskb = np.zeros((8, 128), np.float32)
    for j in range(8):
        maskb[j, 16 * j:16 * (j + 1)] = 1.0
    maskb = maskb.astype(bf)
    # single-tile variants: j = 8*half + GT2COL[gt]*2 + s2 -> 16-col block
    biasTf = np.zeros((16, 128), np.float32)
    for j in range(16):
        h_, r = j // 8, j % 8
        gt = GT2COL.index(r // 2)
        m = gt * 4 + 2 * h_ + (r % 2)
        biasTf[j] = bias_p[128 * m:128 * (m + 1)]
    biasTf = biasTf.astype(bf)
    maskf = np.zeros((16, 256), np.float32)
    for j in range(16):
        maskf[j, 16 * j:16 * (j + 1)] = 1.0
    maskf = maskf.astype(bf)

    cdt = bf if CONV_BF16 else np.float32
    cw = np.zeros((10, 128, 256), np.float32)
    for k in range(5):
        for cc in range(2):
            cw[k * 2 + cc] = conv_w[:, 128 * cc:128 * (cc + 1), k].T
    if EVICT_DVE:
        # fold the BN scale into the conv weights (per output channel =
        # last axis of cw); eviction then only needs (+bias, relu)
        cw = cw * bn_a.astype(np.float32)[None, None, :]
    cw = cw.astype(cdt)

    w1T = np.ascontiguousarray((2.0 * w1).T.reshape(4, 128, 64)).astype(bf)
    w2T = np.ascontiguousarray(w2.T).astype(bf)

    common = dict(
        cw=cw,
        bn_ab=bn_ab,
        wihT=wihT,
        whhT=whhT,
        biasT=biasT,
        maskb=maskb,
        biasTf=biasTf,
        maskf=maskf,
        w1T=w1T,
        b1r=b1.reshape(1, 64).astype(bf),
        w2T=w2T,
        b2r=b2.reshape(1, 2).astype(bf),
        ones1=np.ones((1, 16), bf),
    )
    in_maps = []
    x_c = x.astype(cdt)
    for core in range(NCORES):
        m = dict(common)
        m["x_l"] = np.ascontiguousarray(x_c[BL * core:BL * (core + 1)])
        in_maps.append(m)
    return in_maps


def kernel(**inputs) -> np.ndarray:
    if "nc" not in _cache:
        _cache["nc"] = _build()
    nc = _cache["nc"]
    in_maps = _prep(inputs)
    res = run_bass_kernel_spmd(nc, in_maps, list(range(NCORES)))
    _cache["last_res"] = res
    return np.concatenate([res.results[c]["out"].T for c in range(NCORES)],
                          axis=0).astype(np.float32)

